# revision 21
# baseline (speedup 1.0000x reference)
"""Trainium2 Bass kernel: 6-layer decoder (masked self-attn + cross-attn + FFN).

Sharding (8 cores): 4 batch pairs x 2-way sequence-parallel.
Core r: batch r//2, half g=r%2. Global 512-token chunks: g=0 owns [c0,c3],
g=1 owns [c1,c2] (zigzag for causal load balance). The causal structure is
identical across cores (union schedule); per-core differences are data only
(exp-bias columns and diagonal mask constants).

v2 pipeline: token-chunk software pipeline per layer. The residual stream
lives in bf16 (hbx, both halves); the pair exchanges the own half via a
bf16 AllGather that overlaps the next layer's projections. Scores go to
bf16 PSUM in 2-ktile chunks with one batched exp per chunk; heads are
emitted in even/odd pairs whose score matmuls occupy disjoint PE row
groups (concurrent on HW). Softmax normalization uses DVE fast reciprocal
+ gpsimd partition broadcast. out_proj/LN/FFN work is interleaved into the
attention instruction stream as filler so the PE stays busy during exp.
"""

import os
from collections import deque

import numpy as np
import ml_dtypes

import concourse.bass as bass
import concourse.mybir as mybir
import concourse.tile as tile
from concourse import bacc
from concourse.bass import ts
from concourse.bass_utils import run_bass_kernel_spmd

L, B, S, D, H, DK, F = 6, 4, 2048, 512, 8, 64, 2048
P = 128
TCH = 512                 # token chunk = matmul free dim
HALF = S // 2             # tokens owned per core
KC = D // P               # 4 partition chunks of d_model
NFT = F // P              # 16 feature tiles of FFN hidden
NKT = S // P              # 16 k-tiles over full sequence
AVW = DK + 1              # V columns per head + ones column (softmax sum)
CH = 2                    # k-tiles per scores/exp chunk
EPS = 1e-5
SCALE = 1.0 / float(np.sqrt(DK))
NEG = -1e9

f32 = mybir.dt.float32
f32r = mybir.dt.float32r
bf16 = mybir.dt.bfloat16
AF = mybir.ActivationFunctionType
ALU = mybir.AluOpType

NLAYERS = int(os.environ.get("KERNEL_NLAYERS", str(L)))
OPT_ACTSET = bool(int(os.environ.get("KOPT_ACTSET", "1")))
RG = [[0, 1], [2, 3], [4, 5], [6, 7]]

# Union causal schedule (identical on every core). Local k-tile order:
# 0-3 = my chunk j0, 4-7 = my chunk j1, 8-11 = peer j0, 12-15 = peer j1.
# Entries: (ktile, exp-bias pbias column or None, dmask index or None).
SA_KTS = {
    0: [(0, None, 0), (1, None, 1), (2, None, 2), (3, None, 3),
        (8, 0, None), (9, 0, None), (10, 0, None), (11, 0, None)],
    1: [(0, None, None), (1, None, None), (2, None, None), (3, None, None),
        (4, None, 0), (5, None, 1), (6, None, 2), (7, None, 3),
        (8, 4, None), (9, 4, None), (10, 4, None), (11, 4, None),
        (12, 8, None), (13, 8, None), (14, 8, None), (15, 8, None)],
}
CA_KTS = [(kt, None, None) for kt in range(NKT)]


def _single_act_set():
    # Force every ACT function onto natural_log_exp_and_others (it contains
    # Exp, Ln, Identity and Relu) so the compiled kernel has exactly one
    # ACT_TABLE_LOAD instead of thrashing between per-function sets.
    real = bacc.get_activation_tables

    def patched(arch):
        tabs = real(arch)
        return {name: (fns if name == "natural_log_exp_and_others" else set())
                for name, fns in tabs.items()}

    bacc.get_activation_tables = patched


if OPT_ACTSET:
    _single_act_set()


class FillerQueue:
    """Units of independent work interleaved into attention streams."""

    def __init__(self):
        self.q = deque()

    def add(self, fn):
        self.q.append(fn)

    def pop(self, n=1):
        for _ in range(n):
            if not self.q:
                return
            self.q.popleft()()

    def drain(self):
        while self.q:
            self.q.popleft()()


def build(ln_affine: bool, v_bias: bool):
    nc = bacc.Bacc(None, target_bir_lowering=False, num_devices=8)

    xTb = nc.declare_dram_parameter("xTb", [P, KC, S], bf16, isOutput=False)
    encTb = nc.declare_dram_parameter("encTb", [P, KC, S], bf16, isOutput=False)
    w_in = {}
    for pre in ("sa", "ca"):
        for nm in ("wq", "wk", "wv", "wo"):
            w_in[f"{pre}_{nm}"] = nc.declare_dram_parameter(f"{pre}_{nm}", [L, D, D], bf16, isOutput=False)
        for nm in ("bq", "bk", "bv", "bo"):
            w_in[f"{pre}_{nm}"] = nc.declare_dram_parameter(f"{pre}_{nm}", [L, D], f32, isOutput=False)
    w_in["ff_w1"] = nc.declare_dram_parameter("ff_w1", [L, D, F], bf16, isOutput=False)
    w_in["ff_b1"] = nc.declare_dram_parameter("ff_b1", [L, F], f32, isOutput=False)
    w_in["ff_w2b"] = nc.declare_dram_parameter("ff_w2b", [L, F, D], bf16, isOutput=False)
    w_in["ff_b2"] = nc.declare_dram_parameter("ff_b2", [L, D], f32, isOutput=False)
    if ln_affine:
        for i in (1, 2, 3):
            w_in[f"ln{i}_g"] = nc.declare_dram_parameter(f"ln{i}_g", [L, D], f32, isOutput=False)
            w_in[f"ln{i}_b"] = nc.declare_dram_parameter(f"ln{i}_b", [L, D], f32, isOutput=False)
    ones_in = nc.declare_dram_parameter("ones", [P, P], f32r, isOutput=False)
    onesb_in = nc.declare_dram_parameter("onesb", [P, P], bf16, isOutput=False)
    identb_in = nc.declare_dram_parameter("identb", [P, P], bf16, isOutput=False)
    dmask_in = nc.declare_dram_parameter("dmask", [P, 4, TCH], bf16, isOutput=False)
    pbias_in = nc.declare_dram_parameter("pbias", [P, 12], f32, isOutput=False)
    out_p = nc.declare_dram_parameter("out", [P, KC, HALF], f32, isOutput=True)

    with tile.TileContext(nc, num_cores=8) as tc:
        import contextlib

        gctx = contextlib.ExitStack()
        with gctx:
            persist = gctx.enter_context(tc.tile_pool(name="persist", bufs=1))
            wpool = gctx.enter_context(tc.tile_pool(name="wpool", bufs=1))
            lpool = gctx.enter_context(tc.tile_pool(name="lpool", bufs=1))
            psS = gctx.enter_context(tc.tile_pool(name="psS", bufs=2, space="PSUM"))
            psO = gctx.enter_context(tc.tile_pool(name="psO", bufs=1, space="PSUM"))
            psA = gctx.enter_context(tc.tile_pool(name="psA", bufs=2, space="PSUM"))
            dramp = gctx.enter_context(tc.tile_pool(name="dramp", bufs=2, space="DRAM"))

            # ---- persistent state (SBUF) ----
            hbx = persist.tile([P, KC, HALF], bf16, name="hbx")  # own residual
            kT = persist.tile([P, KC, S], bf16, name="kT")       # shared SA/CA K^T
            vaug = persist.tile([P, NKT, H, AVW], bf16, name="vaug")  # shared aug-V
            oT_s = persist.tile([P, KC, HALF], bf16, name="oT_s")
            oT_c = persist.tile([P, KC, HALF], bf16, name="oT_c")
            x1b = persist.tile([P, KC, HALF], bf16, name="x1b")
            yT = persist.tile([P, KC, HALF], bf16, name="yT")
            h1 = persist.tile([P, NFT, TCH], bf16, name="h1")
            u_t = persist.tile([P, KC, TCH], f32r, name="u_t")   # psum-evac target

            ones_sb = persist.tile([P, P], f32r, name="ones_sb")
            onesb_sb = persist.tile([P, P], bf16, name="onesb_sb")
            identb_sb = persist.tile([P, P], bf16, name="identb_sb")
            dmask_sb = persist.tile([P, 4, TCH], bf16, name="dmask_sb")
            pbias_sb = persist.tile([P, 12], f32, name="pbias_sb")
            zero_sb = persist.tile([P, 1], f32, name="zero_sb")
            eps_sb = persist.tile([P, 1], f32, name="eps_sb")
            nc.vector.memset(zero_sb, 0.0)
            nc.vector.memset(eps_sb, EPS)
            # ones columns of the augmented-V layout, set once (V writes
            # never touch them, across all layers and both attentions)
            nc.vector.memset(vaug[:, :, :, DK:DK + 1], 1.0)

            for kc in range(KC):
                nc.sync.dma_start(out=hbx[:, kc, :], in_=xTb[:, kc, 0:HALF])
            nc.sync.dma_start(out=ones_sb, in_=ones_in[:, :])
            nc.sync.dma_start(out=onesb_sb, in_=onesb_in[:, :])
            nc.sync.dma_start(out=identb_sb, in_=identb_in[:, :])
            nc.sync.dma_start(out=dmask_sb, in_=dmask_in[:, :, :])
            nc.sync.dma_start(out=pbias_sb, in_=pbias_in[:, :])

            pid = nc.sync.partition_id()
            peer = (pid + 1) % 2

            def load_w(dram_t, l, cols, tag, bufs=2):
                n = dram_t.shape[1] // P
                l = l % L
                t = wpool.tile([P, n, cols], bf16, tag=tag, bufs=bufs, name=tag)
                for kc in range(n):
                    nc.sync.dma_start(out=t[:, kc, :], in_=dram_t[l, kc * P:(kc + 1) * P, :])
                return t

            def load_b(dram_t, l, tag):
                n = dram_t.shape[1] // P
                l = l % L
                t = wpool.tile([P, n], f32, tag=tag, bufs=2, name=tag)
                nc.sync.dma_start(out=t, in_=dram_t[l].rearrange("(c p) -> p c", p=P))
                return t

            def evac(dst, src_ps, bias_col, eng):
                if eng == "act":
                    nc.scalar.activation(dst, src_ps, AF.Identity, bias=bias_col)
                else:
                    nc.vector.tensor_scalar(dst, src_ps, bias_col, None, ALU.add)

            # ---------------- building blocks ----------------

            def kv_chunk(src, t, wk_sb, bk_sb, wv_sb, bv_sb, eng):
                """K^T + augmented V for one 512-token chunk t (bf16)."""
                for ft in range(KC):
                    k_ps = psA.tile([P, TCH], f32, tag="acc", name="k_ps")
                    for kc in range(KC):
                        nc.tensor.matmul(k_ps, wk_sb[:, kc, ft * P:(ft + 1) * P],
                                         src[:, kc, :], start=(kc == 0), stop=(kc == KC - 1))
                    evac(kT[:, ft, t * TCH:(t + 1) * TCH], k_ps, bk_sb[:, ft:ft + 1], eng)
                for tl in range(4):
                    tt = t * 4 + tl
                    v_ps = psA.tile([P, D], f32, tag="acc", name="v_ps")
                    nmm = KC + (1 if v_bias else 0)
                    for kc in range(KC):
                        nc.tensor.matmul(v_ps, src[:, kc, tl * P:(tl + 1) * P],
                                         wv_sb[:, kc, :], start=(kc == 0),
                                         stop=(kc == nmm - 1))
                    if v_bias:
                        nc.tensor.matmul(v_ps, onesb_sb[0:1, :], bv_sb, start=False, stop=True)
                    # one strided copy drops all 8 heads into the aug layout
                    nc.vector.tensor_copy(
                        out=vaug[:, tt, :, 0:DK],
                        in_=v_ps.rearrange("p (h d) -> p h d", h=H))

            def q_pair(srcT, j, hp, wq_sb, bq_sb, eng):
                """One head pair's Q for query chunk j -> transient tile."""
                qp = lpool.tile([P, TCH], bf16, tag="qp", bufs=2, name="qp")
                q_ps = psA.tile([P, TCH], f32, tag="acc", name="q_ps")
                for kc in range(KC):
                    nc.tensor.matmul(q_ps, wq_sb[:, kc, hp * P:(hp + 1) * P],
                                     srcT[:, kc, j * TCH:(j + 1) * TCH],
                                     start=(kc == 0), stop=(kc == KC - 1))
                evac(qp, q_ps, bq_sb[:, hp:hp + 1], eng)
                return qp

            def attn_pair_j(hp, j, kts, q_src, oT_t, fillers):
                """Attention for head pair (2hp, 2hp+1), query chunk j.

                Per k-tile: the two heads' score matmuls target partition
                offsets 0/64 (disjoint PE row groups -> concurrent on HW)
                and land in the two banks of one [P, 2, TCH] f32 PSUM
                tile; a single batched exp covers both heads."""
                hA, hB = 2 * hp, 2 * hp + 1
                qp = q_src(hp, j)
                o_ps = {}
                for hx in (0, 1):
                    o_ps[hx] = psO.tile([AVW, TCH], f32, tag=f"o{hx}", name=f"o_ps{hx}")
                nkt_total = len(kts)
                for done, (kt, bcol, diag) in enumerate(kts):
                    s_ps = psS.tile([P, 2, TCH], f32, tag="s", name="s_ps")
                    pt = lpool.tile([P, 2, TCH], bf16, tag="pt", bufs=2, name="pt")
                    for hx, h in ((0, hA), (1, hB)):
                        off = (h % 2) * DK
                        nc.tensor.matmul(
                            s_ps[:, hx, :],
                            kT[off:off + DK, h // 2, kt * P:(kt + 1) * P],
                            qp[off:off + DK, :],
                            start=True, stop=True)
                    bias = zero_sb[:, 0:1] if bcol is None else pbias_sb[:, bcol:bcol + 1]
                    nc.scalar.activation(pt, s_ps, AF.Exp, bias=bias, scale=SCALE)
                    if diag is not None:
                        # all-bf16 SBUF operands -> DVE 4x mode (~200ns)
                        for hx in (0, 1):
                            nc.vector.tensor_mul(pt[:, hx, :], pt[:, hx, :],
                                                 dmask_sb[:, diag, :])
                    for hx, h in ((0, hA), (1, hB)):
                        nc.tensor.matmul(o_ps[hx], vaug[:, kt, h, :],
                                         pt[:, hx, :], start=(done == 0),
                                         stop=(done == nkt_total - 1))
                    if done % 2 == 1:
                        fillers.pop(1)
                # softmax normalization: fast reciprocal + partition broadcast
                for hx, h in ((0, hA), (1, hB)):
                    off = (h % 2) * DK
                    srow = lpool.tile([1, TCH], f32, tag="srow", bufs=2, name="srow")
                    nc.vector.tensor_copy(out=srow, in_=o_ps[hx][DK:AVW, :])
                    rcp = lpool.tile([1, TCH], f32, tag="rcp", bufs=2, name="rcp")
                    nc.vector.reciprocal_approx_fast(out=rcp, in_=srow)
                    rb = lpool.tile([DK, TCH], f32, tag="rb", bufs=1, name="rb")
                    nc.gpsimd.partition_broadcast(rb, rcp, channels=DK)
                    nc.vector.tensor_mul(
                        oT_t[off:off + DK, h // 2, j * TCH:(j + 1) * TCH],
                        o_ps[hx][0:DK, :], rb)

            def out_proj_ft(j, ft, oT_t, wo_sb, bo_sb, eng):
                u_ps = psA.tile([P, TCH], f32, tag="acc", name="u_ps")
                for kc in range(KC):
                    nc.tensor.matmul(u_ps, wo_sb[:, kc, ft * P:(ft + 1) * P],
                                     oT_t[:, kc, j * TCH:(j + 1) * TCH],
                                     start=(kc == 0), stop=False)
                # residual from the bf16 stream via identity matmul
                nc.tensor.matmul(u_ps, identb_sb, hbx[:, ft, j * TCH:(j + 1) * TCH],
                                 start=False, stop=True)
                evac(u_t[:, ft, :], u_ps, bo_sb[:, ft:ft + 1], eng)

            def layernorm_t(dsts, g_sb, b_sb):
                """LN over u_t -> dsts(kc)."""
                usq = lpool.tile([P, KC, TCH], bf16, tag="usq", bufs=1, name="usq")
                for kc in range(KC):
                    nc.gpsimd.tensor_mul(usq[:, kc, :], u_t[:, kc, :], u_t[:, kc, :])
                m_ps = psA.tile([P, TCH], f32, tag="acc", name="m_ps")
                for kc in range(KC):
                    nc.tensor.matmul(m_ps, ones_sb, u_t[:, kc, :],
                                     start=(kc == 0), stop=(kc == KC - 1))
                q_ps = psA.tile([P, TCH], f32, tag="acc", name="q_ps")
                for kc in range(KC):
                    nc.tensor.matmul(q_ps, onesb_sb, usq[:, kc, :],
                                     start=(kc == 0), stop=(kc == KC - 1))
                t_sb = lpool.tile([P, TCH], f32, tag="lnr", bufs=3, name="t_sb")
                nc.vector.tensor_scalar(t_sb, m_ps, 1.0 / D, None, ALU.mult)
                m2 = lpool.tile([P, TCH], f32, tag="lnr", bufs=3, name="m2")
                nc.vector.tensor_scalar(m2, q_ps, 1.0 / D, None, ALU.mult)
                tt2 = lpool.tile([P, TCH], f32, tag="lnr", bufs=3, name="tt2")
                nc.gpsimd.tensor_mul(tt2, t_sb, t_sb)
                nc.vector.tensor_sub(m2, m2, tt2)
                nc.scalar.activation(m2, m2, AF.Ln, bias=eps_sb[:, 0:1])
                r_sb = lpool.tile([P, TCH], f32, tag="lnr", bufs=3, name="r_sb")
                nc.scalar.activation(r_sb, m2, AF.Exp, scale=-0.5, bias=zero_sb[:, 0:1])
                c_sb = lpool.tile([P, TCH], f32, tag="lnc", bufs=1, name="c_sb")
                nc.vector.tensor_mul(c_sb, t_sb, r_sb)
                for kc in range(KC):
                    tmp = lpool.tile([P, TCH], f32, tag="ltmp", bufs=1, name="ltmp")
                    nc.gpsimd.tensor_sub(tmp, u_t[:, kc, :], c_sb)
                    d = dsts(kc)
                    nc.vector.tensor_mul(d, tmp, r_sb)
                    if ln_affine:
                        nc.vector.tensor_scalar(d, d, g_sb[:, kc:kc + 1], b_sb[:, kc:kc + 1],
                                                ALU.mult, ALU.add)

            # ---------------- layer loop ----------------
            pending_fetch = []
            for l in range(NLAYERS):
                last = l == NLAYERS - 1

                # ---- SA projections (ACT idle here -> act evacs) ----
                wk_sa = load_w(w_in["sa_wk"], l, D, "wk", bufs=2)
                wv_sa = load_w(w_in["sa_wv"], l, D, "wv")
                bk_sa = load_b(w_in["sa_bk"], l, "bk")
                bq_sa = load_b(w_in["sa_bq"], l, "bq")
                bv_sa = None
                if v_bias:
                    bv_sa = wpool.tile([1, D], f32, tag="bv", bufs=2, name="bv")
                    nc.sync.dma_start(out=bv_sa, in_=w_in["sa_bv"][l % L:l % L + 1, :])
                wq_sa = load_w(w_in["sa_wq"], l, D, "wq")

                def hbx_chunk(t):
                    return hbx[:, :, t * TCH:(t + 1) * TCH]

                def peer_chunk(half):
                    pc = lpool.tile([P, KC, TCH], bf16, tag="peer", bufs=1,
                                    name="peerC")
                    if l == 0:
                        for kc in range(KC):
                            nc.sync.dma_start(
                                out=pc[:, kc, :],
                                in_=xTb[:, kc, HALF + half * TCH:HALF + (half + 1) * TCH])
                    else:
                        ccout = pending_fetch[half]
                        nc.sync.dma_start(out=pc, in_=ccout[ts(peer, P), :, :])
                    return pc

                for t in (0, 1):
                    kv_chunk(hbx_chunk(t), t, wk_sa, bk_sa, wv_sa, bv_sa, "act")
                # peer chunk j0 (waits on gather half 0 from the prev layer)
                kv_chunk(peer_chunk(0), 2, wk_sa, bk_sa, wv_sa, bv_sa, "act")

                wo_sa = load_w(w_in["sa_wo"], l, D, "wo")
                bo_sa = load_b(w_in["sa_bo"], l, "bo")
                g1 = load_b(w_in["ln1_g"], l, "g1") if ln_affine else None
                b1l = load_b(w_in["ln1_b"], l, "b1l") if ln_affine else None
                wq_ca = load_w(w_in["ca_wq"], l, D, "wq2")
                bq_ca = load_b(w_in["ca_bq"], l, "bq2")

                # ---- SA attention ----
                # j0 touches only own chunks + peer j0 (gather half 0), so it
                # runs while gather half 1 is still in flight; peer chunk j1
                # projection follows it.
                fill = FillerQueue()
                def sa_q(hp, j):
                    return q_pair(hbx, j, hp, wq_sa, bq_sa, "dve")

                for hp in range(4):
                    attn_pair_j(hp, 0, SA_KTS[0], sa_q, oT_s, fill)
                kv_chunk(peer_chunk(1), 3, wk_sa, bk_sa, wv_sa, bv_sa, "act")
                pending_fetch.clear()
                # fillers for j1: out_proj j0 + LN1 t0
                for ft in range(KC):
                    fill.add(lambda ft=ft: out_proj_ft(0, ft, oT_s, wo_sa, bo_sa, "dve"))
                fill.add(lambda: layernorm_t(
                    lambda kc: x1b[:, kc, 0 * TCH:1 * TCH], g1, b1l))
                for hp in range(4):
                    attn_pair_j(hp, 1, SA_KTS[1], sa_q, oT_s, fill)
                fill.drain()

                # ---- CA K/V from enc (shared kT/vaug buffers) ----
                wk_ca = load_w(w_in["ca_wk"], l, D, "wk")
                wv_ca = load_w(w_in["ca_wv"], l, D, "wv")
                bk_ca = load_b(w_in["ca_bk"], l, "bk")
                bv_ca = None
                if v_bias:
                    bv_ca = wpool.tile([1, D], f32, tag="bv", bufs=2, name="bv")
                    nc.sync.dma_start(out=bv_ca, in_=w_in["ca_bv"][l % L:l % L + 1, :])
                for t in range(4):
                    ec = lpool.tile([P, KC, TCH], bf16, tag="peer", bufs=1, name="encC")
                    for kc in range(KC):
                        nc.sync.dma_start(out=ec[:, kc, :],
                                          in_=encTb[:, kc, t * TCH:(t + 1) * TCH])
                    kv_chunk(ec, t, wk_ca, bk_ca, wv_ca, bv_ca, "dve")

                wo_ca = load_w(w_in["ca_wo"], l, D, "wo")
                bo_ca = load_b(w_in["ca_bo"], l, "bo")
                g2 = load_b(w_in["ln2_g"], l, "g2") if ln_affine else None
                b2l = load_b(w_in["ln2_b"], l, "b2l") if ln_affine else None

                # ---- CA attention ----
                def ca_q(hp, j):
                    return q_pair(x1b, j, hp, wq_ca, bq_ca, "dve")

                fill = FillerQueue()
                # fillers for j0: SA out_proj j1 + LN1 t1
                for ft in range(KC):
                    fill.add(lambda ft=ft: out_proj_ft(1, ft, oT_s, wo_sa, bo_sa, "dve"))
                fill.add(lambda: layernorm_t(
                    lambda kc: x1b[:, kc, 1 * TCH:2 * TCH], g1, b1l))
                for hp in range(4):
                    attn_pair_j(hp, 0, CA_KTS, ca_q, oT_c, fill)
                fill.drain()

                w1_sb = load_w(w_in["ff_w1"], l, F, "w1", bufs=1)
                b1_sb = load_b(w_in["ff_b1"], l, "b1")
                w2_sb = wpool.tile([P, NFT, D], bf16, tag="w2", bufs=1, name="w2_sb")
                for kc in range(NFT):
                    nc.sync.dma_start(out=w2_sb[:, kc, :],
                                      in_=w_in["ff_w2b"][l % L, kc * P:(kc + 1) * P, :])
                b2_sb = load_b(w_in["ff_b2"], l, "b2")
                g3 = load_b(w_in["ln3_g"], l, "g3") if ln_affine else None
                b3l = load_b(w_in["ln3_b"], l, "b3l") if ln_affine else None

                def ca_post_j(j):
                    for ft in range(KC):
                        out_proj_ft(j, ft, oT_c, wo_ca, bo_ca, "dve")
                    layernorm_t(lambda kc, j=j: yT[:, kc, j * TCH:(j + 1) * TCH],
                                g2, b2l)

                def ffn1_ft(t, ft):
                    f_ps = psA.tile([P, TCH], f32, tag="acc", name="f_ps")
                    for kc in range(KC):
                        nc.tensor.matmul(f_ps, w1_sb[:, kc, ft * P:(ft + 1) * P],
                                         yT[:, kc, t * TCH:(t + 1) * TCH],
                                         start=(kc == 0), stop=(kc == KC - 1))
                    nc.vector.tensor_scalar(h1[:, ft, :], f_ps,
                                            b1_sb[:, ft:ft + 1], 0.0, ALU.add, ALU.max)

                def ffn1_t(t):
                    for ft in range(NFT):
                        ffn1_ft(t, ft)

                def ffn2_ft(t, ft):
                    g_ps = psA.tile([P, TCH], f32, tag="acc", name="g_ps")
                    for kc in range(NFT):
                        nc.tensor.matmul(g_ps, w2_sb[:, kc, ft * P:(ft + 1) * P],
                                         h1[:, kc, :],
                                         start=(kc == 0), stop=False)
                    nc.tensor.matmul(g_ps, identb_sb,
                                     x1b[:, ft, t * TCH:(t + 1) * TCH],
                                     start=False, stop=True)
                    evac(u_t[:, ft, :], g_ps, b2_sb[:, ft:ft + 1], "act")

                def ffn2_t(t):
                    for ft in range(KC):
                        ffn2_ft(t, ft)

                def ln3_t(t):
                    if last:
                        # write LN output in place over u_t, then DMA it out
                        layernorm_t(lambda kc: u_t[:, kc, :], g3, b3l)
                        nc.sync.dma_start(out=out_p[:, :, t * TCH:(t + 1) * TCH],
                                          in_=u_t.bitcast(f32))
                    else:
                        layernorm_t(lambda kc, t=t: hbx[:, kc, t * TCH:(t + 1) * TCH],
                                    g3, b3l)

                def issue_gather(half):
                    """Start the AllGather for own chunk `half`; the peer's
                    copy is fetched into a transient tile next layer."""
                    ccin = dramp.tile([P, KC, TCH], bf16, tag="ccin", bufs=2,
                                      name="ccin")
                    ccout = dramp.tile([2 * P, KC, TCH], bf16, tag="ccout", bufs=2,
                                       name="ccout")
                    nc.sync.dma_start(out=ccin, in_=hbx[:, :, half * TCH:(half + 1) * TCH])
                    nc.gpsimd.collective_compute(
                        "AllGather", ALU.bypass, replica_groups=RG,
                        ins=[ccin.opt()], outs=[ccout.opt()])
                    return ccout

                fill = FillerQueue()
                for ft in range(KC):
                    fill.add(lambda ft=ft: out_proj_ft(0, ft, oT_c, wo_ca, bo_ca, "dve"))
                fill.add(lambda: layernorm_t(
                    lambda kc: yT[:, kc, 0 * TCH:1 * TCH], g2, b2l))
                for ft in range(NFT):
                    fill.add(lambda ft=ft: ffn1_ft(0, ft))
                # chunk-0 FFN2/LN3 + first gather go in as fillers too, so the
                # collective starts as early as possible
                for ft in range(KC):
                    fill.add(lambda ft=ft: ffn2_ft(0, ft))
                fill.add(lambda: ln3_t(0))
                if not last:
                    fill.add(lambda: pending_fetch.append(issue_gather(0)))
                for hp in range(4):
                    attn_pair_j(hp, 1, CA_KTS, ca_q, oT_c, fill)
                fill.drain()

                ca_post_j(1)
                ffn1_t(1)
                ffn2_t(1)
                ln3_t(1)
                if not last:
                    pending_fetch.append(issue_gather(1))
                    assert len(pending_fetch) == 2

    nc.finalize()
    return nc


_BUILD_CACHE = {}
LAST_RESULTS = None


def _get_nc(ln_affine, v_bias):
    key = (ln_affine, v_bias, NLAYERS, OPT_ACTSET)
    if key not in _BUILD_CACHE:
        _BUILD_CACHE[key] = build(ln_affine, v_bias)
    return _BUILD_CACHE[key]


def _to_T(a, dtype):  # [S, D] -> [P, KC, S] feature-major
    return np.ascontiguousarray(a.T.reshape(KC, P, S).transpose(1, 0, 2)).astype(dtype)


def prepare(inputs):
    """Returns (nc, in_maps) for the given full inputs."""
    inp = {k: np.asarray(v) for k, v in inputs.items()}

    ln_affine = not all(
        np.all(inp[f"ln{i}_g"] == 1.0) and np.all(inp[f"ln{i}_b"] == 0.0) for i in (1, 2, 3)
    )
    v_bias = not (np.all(inp["sa_bv"] == 0.0) and np.all(inp["ca_bv"] == 0.0))
    nc = _get_nc(ln_affine, v_bias)

    ident = np.eye(P, dtype=np.float32)
    pcol = np.arange(P)[:, None]
    qcol = np.arange(TCH)[None, :]
    dmask = np.stack(
        [(qcol >= i * P + pcol) for i in range(4)], axis=1
    ).astype(ml_dtypes.bfloat16)  # [P, 4, TCH]

    shared = {}
    for pre in ("sa", "ca"):
        for nm in ("bq", "bk", "bv", "bo"):
            shared[f"{pre}_{nm}"] = np.ascontiguousarray(inp[f"{pre}_{nm}"], np.float32)
        for nm in ("wq", "wk", "wv", "wo"):
            shared[f"{pre}_{nm}"] = inp[f"{pre}_{nm}"].astype(ml_dtypes.bfloat16)
    shared["ff_w1"] = inp["ff_w1"].astype(ml_dtypes.bfloat16)
    shared["ff_b1"] = np.ascontiguousarray(inp["ff_b1"], np.float32)
    shared["ff_w2b"] = inp["ff_w2"].astype(ml_dtypes.bfloat16)
    shared["ff_b2"] = np.ascontiguousarray(inp["ff_b2"], np.float32)
    if ln_affine:
        for i in (1, 2, 3):
            shared[f"ln{i}_g"] = np.ascontiguousarray(inp[f"ln{i}_g"], np.float32)
            shared[f"ln{i}_b"] = np.ascontiguousarray(inp[f"ln{i}_b"], np.float32)
    shared["ones"] = np.ones((P, P), np.float32)
    shared["onesb"] = np.ones((P, P), np.float32).astype(ml_dtypes.bfloat16)
    shared["identb"] = ident.astype(ml_dtypes.bfloat16)
    shared["dmask"] = dmask

    in_maps = []
    for r in range(8):
        b, g = r // 2, r % 2
        mine = [0, 3] if g == 0 else [1, 2]
        theirs = [1, 2] if g == 0 else [0, 3]
        perm = mine + theirs
        xt = np.concatenate([inp["x"][b].T[:, c * TCH:(c + 1) * TCH] for c in perm], axis=1)
        xt = np.ascontiguousarray(xt.reshape(KC, P, S).transpose(1, 0, 2))
        m = dict(shared)
        m["xTb"] = xt.astype(ml_dtypes.bfloat16)
        m["encTb"] = _to_T(np.asarray(inp["enc"][b], np.float32), ml_dtypes.bfloat16)
        pb = np.zeros(12, np.float32)
        # exp-bias columns: j0 kt8-11 -> 0..3 ; j1 kt8-11 -> 4..7 ; j1 kt12-15 -> 8..11
        # Each group of 4 k-tiles lies in one peer global chunk kg; keep iff kg < qg.
        for base, j, kg in ((0, 0, theirs[0]), (4, 1, theirs[0]), (8, 1, theirs[1])):
            pb[base:base + 4] = 0.0 if kg < mine[j] else NEG
        m["pbias"] = np.broadcast_to(pb, (P, 12)).astype(np.float32).copy()
        in_maps.append(m)
    return nc, in_maps


def unshard(results):
    out = np.zeros((B, S, D), np.float32)
    for r in range(8):
        b, g = r // 2, r % 2
        mine = [0, 3] if g == 0 else [1, 2]
        half = results[r]["out"].transpose(1, 0, 2).reshape(D, HALF)
        for j, c in enumerate(mine):
            out[b, c * TCH:(c + 1) * TCH, :] = half[:, j * TCH:(j + 1) * TCH].T
    return out


def kernel(**inputs):
    global LAST_RESULTS
    nc, in_maps = prepare(inputs)

    res = None
    for attempt in range(3):
        try:
            res = run_bass_kernel_spmd(
                nc, in_maps, core_ids=list(range(8)),
                trace=bool(int(os.environ.get("KERNEL_TRACE", "0"))),
            )
            break
        except Exception:
            # first execution after a fresh NEFF compile occasionally flakes
            # on the runtime side; the NEFF cache makes the retry cheap
            if attempt == 2:
                raise
    LAST_RESULTS = res
    return unshard(res.results)


# revision 51
# speedup vs baseline: 1.1790x; 1.1790x over previous
"""Trainium2 Bass kernel: 6-layer decoder (masked self-attn + cross-attn + FFN).

Sharding (8 cores): 4 batch pairs x 2-way sequence-parallel.
Core r: batch r//2, half g=r%2. Global 512-token chunks: g=0 owns [c0,c3],
g=1 owns [c1,c2] (zigzag for causal load balance). The causal structure is
identical across cores (union schedule); per-core differences are data only
(exp-bias columns and diagonal mask constants).

v2 pipeline: token-chunk software pipeline per layer. The residual stream
lives in bf16 (hbx, both halves); the pair exchanges the own half via a
bf16 AllGather that overlaps the next layer's projections. Scores go to
bf16 PSUM in 2-ktile chunks with one batched exp per chunk; heads are
emitted in even/odd pairs whose score matmuls occupy disjoint PE row
groups (concurrent on HW). Softmax normalization uses DVE fast reciprocal
+ gpsimd partition broadcast. out_proj/LN/FFN work is interleaved into the
attention instruction stream as filler so the PE stays busy during exp.
"""

import os
from collections import deque

import numpy as np
import ml_dtypes

import concourse.bass as bass
import concourse.mybir as mybir
import concourse.tile as tile
from concourse import bacc
from concourse.bass import ts
from concourse.bass_utils import run_bass_kernel_spmd

L, B, S, D, H, DK, F = 6, 4, 2048, 512, 8, 64, 2048
P = 128
TCH = 512                 # token chunk = matmul free dim
HALF = S // 2             # tokens owned per core
KC = D // P               # 4 partition chunks of d_model
NFT = F // P              # 16 feature tiles of FFN hidden
NKT = S // P              # 16 k-tiles over full sequence
AVW = DK + 1              # V columns per head + ones column (softmax sum)
CH = 2                    # k-tiles per scores/exp chunk
W2SCL = 16.0              # host pre-scale of fp8 W2 (keeps it out of denormals)
EPS = 1e-5
SCALE = 1.0 / float(np.sqrt(DK))
NEG = -1e9

f32 = mybir.dt.float32
f32r = mybir.dt.float32r
bf16 = mybir.dt.bfloat16
fp8 = mybir.dt.float8e4
AF = mybir.ActivationFunctionType
ALU = mybir.AluOpType

NLAYERS = int(os.environ.get("KERNEL_NLAYERS", str(L)))
OPT_ACTSET = bool(int(os.environ.get("KOPT_ACTSET", "1")))
RG = [[0, 1], [2, 3], [4, 5], [6, 7]]

# Union causal schedule (identical on every core). Local k-tile order:
# 0-3 = my chunk j0, 4-7 = my chunk j1, 8-11 = peer j0, 12-15 = peer j1.
# Entries: (ktile, exp-bias pbias column or None, dmask index or None).
SA_KTS = {
    0: [(0, None, 0), (1, None, 1), (2, None, 2), (3, None, 3),
        (8, 0, None), (9, 0, None), (10, 0, None), (11, 0, None)],
    1: [(0, None, None), (1, None, None), (2, None, None), (3, None, None),
        (4, None, 0), (5, None, 1), (6, None, 2), (7, None, 3),
        (8, 4, None), (9, 4, None), (10, 4, None), (11, 4, None),
        (12, 8, None), (13, 8, None), (14, 8, None), (15, 8, None)],
}
CA_KTS = [(kt, None, None) for kt in range(NKT)]


def _single_act_set():
    # Force every ACT function onto natural_log_exp_and_others (it contains
    # Exp, Ln, Identity and Relu) so the compiled kernel has exactly one
    # ACT_TABLE_LOAD instead of thrashing between per-function sets.
    real = bacc.get_activation_tables

    def patched(arch):
        tabs = real(arch)
        return {name: (fns if name == "natural_log_exp_and_others" else set())
                for name, fns in tabs.items()}

    bacc.get_activation_tables = patched


if OPT_ACTSET:
    _single_act_set()


class FillerQueue:
    """Units of independent work interleaved into attention streams."""

    def __init__(self):
        self.q = deque()

    def add(self, fn):
        self.q.append(fn)

    def pop(self, n=1):
        for _ in range(n):
            if not self.q:
                return
            fn = self.q.popleft()
            if fn is not None:
                fn()

    def drain(self):
        while self.q:
            self.q.popleft()()


def build(ln_affine: bool, v_bias: bool, ffn_bias: bool = False,
          attn_obias: bool = False):
    nc = bacc.Bacc(None, target_bir_lowering=False, num_devices=8)

    xTb = nc.declare_dram_parameter("xTb", [P, KC, S], bf16, isOutput=False)
    encTb = nc.declare_dram_parameter("encTb", [P, KC, S], bf16, isOutput=False)
    w_in = {}
    for pre in ("sa", "ca"):
        for nm in ("wq", "wk", "wv"):
            w_in[f"{pre}_{nm}"] = nc.declare_dram_parameter(f"{pre}_{nm}", [L, D, D], bf16, isOutput=False)
        w_in[f"{pre}_wo"] = nc.declare_dram_parameter(f"{pre}_wo", [L, D, D], bf16, isOutput=False)
        for nm in ("bq", "bk", "bv", "bo"):
            w_in[f"{pre}_{nm}"] = nc.declare_dram_parameter(f"{pre}_{nm}", [L, D], f32, isOutput=False)
    w_in["ff_w1"] = nc.declare_dram_parameter("ff_w1", [L, D, F], bf16, isOutput=False)
    w_in["ff_b1"] = nc.declare_dram_parameter("ff_b1", [L, F], f32, isOutput=False)
    w_in["ff_w2b"] = nc.declare_dram_parameter("ff_w2b", [L, F, D], bf16, isOutput=False)
    w_in["ff_b2"] = nc.declare_dram_parameter("ff_b2", [L, D], f32, isOutput=False)
    if ln_affine:
        for i in (1, 2, 3):
            w_in[f"ln{i}_g"] = nc.declare_dram_parameter(f"ln{i}_g", [L, D], f32, isOutput=False)
            w_in[f"ln{i}_b"] = nc.declare_dram_parameter(f"ln{i}_b", [L, D], f32, isOutput=False)
    onesb_in = nc.declare_dram_parameter("onesb", [P, P], bf16, isOutput=False)
    dmask_in = nc.declare_dram_parameter("dmask", [P, 4, TCH], bf16, isOutput=False)
    pbias_in = nc.declare_dram_parameter("pbias", [P, 12], f32, isOutput=False)
    out_p = nc.declare_dram_parameter("out", [P, KC, HALF], f32, isOutput=True)

    with tile.TileContext(nc, num_cores=8) as tc:
        import contextlib

        gctx = contextlib.ExitStack()
        with gctx:
            persist = gctx.enter_context(tc.tile_pool(name="persist", bufs=1))
            wpool = gctx.enter_context(tc.tile_pool(name="wpool", bufs=1))
            lpool = gctx.enter_context(tc.tile_pool(name="lpool", bufs=1))
            psS = gctx.enter_context(tc.tile_pool(name="psS", bufs=2, space="PSUM"))
            psO = gctx.enter_context(tc.tile_pool(name="psO", bufs=1, space="PSUM"))
            psA = gctx.enter_context(tc.tile_pool(name="psA", bufs=2, space="PSUM"))
            dramp = gctx.enter_context(tc.tile_pool(name="dramp", bufs=2, space="DRAM"))

            # ---- persistent state (SBUF) ----
            hbx = persist.tile([P, KC, HALF], bf16, name="hbx")  # own residual
            kT = persist.tile([P, KC, S], bf16, name="kT")       # shared SA/CA K^T
            kT_c = kT
            vaug = persist.tile([P, NKT, H, AVW], bf16, name="vaug")  # shared aug-V
            oT_s = persist.tile([P, KC, HALF], bf16, name="oT_s")
            oT_c = persist.tile([P, KC, HALF], bf16, name="oT_c")
            x1b = persist.tile([P, KC, HALF], bf16, name="x1b")
            yT = persist.tile([P, KC, HALF], bf16, name="yT")
            h1 = persist.tile([P, NFT, TCH], bf16, name="h1")
            u_t = persist.tile([P, KC, TCH], bf16, name="u_t")   # psum-evac target

            onesb_sb = persist.tile([P, P], bf16, name="onesb_sb")
            dmask_sb = persist.tile([P, 4, TCH], bf16, name="dmask_sb")
            pbias_sb = persist.tile([P, 12], f32, name="pbias_sb")
            zero_sb = persist.tile([P, 1], f32, name="zero_sb")
            eps_sb = persist.tile([P, 1], f32, name="eps_sb")
            nc.vector.memset(zero_sb, 0.0)
            nc.vector.memset(eps_sb, EPS)
            # ones columns of the augmented-V layout, set once (V writes
            # never touch them, across all layers and both attentions)
            nc.vector.memset(vaug[:, :, :, DK:DK + 1], 1.0)

            for kc in range(KC):
                nc.sync.dma_start(out=hbx[:, kc, :], in_=xTb[:, kc, 0:HALF])
            nc.sync.dma_start(out=onesb_sb, in_=onesb_in[:, :])
            nc.sync.dma_start(out=dmask_sb, in_=dmask_in[:, :, :])
            nc.sync.dma_start(out=pbias_sb, in_=pbias_in[:, :])

            pid = nc.sync.partition_id()
            peer = (pid + 1) % 2

            def load_w(dram_t, l, cols, tag, bufs=2, dt=bf16):
                n = dram_t.shape[1] // P
                l = l % L
                t = wpool.tile([P, n, cols], dt, tag=tag, bufs=bufs, name=tag)
                for kc in range(n):
                    nc.sync.dma_start(out=t[:, kc, :], in_=dram_t[l, kc * P:(kc + 1) * P, :])
                return t

            def load_b(dram_t, l, tag):
                n = dram_t.shape[1] // P
                l = l % L
                t = wpool.tile([P, n], f32, tag=tag, bufs=2, name=tag)
                nc.sync.dma_start(out=t, in_=dram_t[l].rearrange("(c p) -> p c", p=P))
                return t

            def evac(dst, src_ps, bias_col, eng):
                if eng == "act":
                    nc.scalar.activation(dst, src_ps, AF.Identity, bias=bias_col)
                else:
                    nc.vector.tensor_scalar(dst, src_ps, bias_col, None, ALU.add)

            # ---------------- building blocks ----------------

            def k_chunk(src, t, wk_sb, bk_sb, kT_t, eng):
                for ft in range(KC):
                    k_ps = psA.tile([P, TCH], f32, tag="acc", name="k_ps")
                    for kc in range(KC):
                        nc.tensor.matmul(k_ps, wk_sb[:, kc, ft * P:(ft + 1) * P],
                                         src[:, kc, :], start=(kc == 0), stop=(kc == KC - 1))
                    evac(kT_t[:, ft, t * TCH:(t + 1) * TCH], k_ps, bk_sb[:, ft:ft + 1], eng)

            def kv_chunk(src, t, wk_sb, bk_sb, wv_sb, bv_sb, eng):
                """K^T + augmented V for one 512-token chunk t (bf16)."""
                k_chunk(src, t, wk_sb, bk_sb, kT, eng)
                for tl in range(4):
                    tt = t * 4 + tl
                    v_ps = psA.tile([P, D], f32, tag="acc", name="v_ps")
                    nmm = KC + (1 if v_bias else 0)
                    for kc in range(KC):
                        nc.tensor.matmul(v_ps, src[:, kc, tl * P:(tl + 1) * P],
                                         wv_sb[:, kc, :], start=(kc == 0),
                                         stop=(kc == nmm - 1))
                    if v_bias:
                        nc.tensor.matmul(v_ps, onesb_sb[0:1, :], bv_sb, start=False, stop=True)
                    # one strided copy drops all 8 heads into the aug layout
                    nc.vector.tensor_copy(
                        out=vaug[:, tt, :, 0:DK],
                        in_=v_ps.rearrange("p (h d) -> p h d", h=H))

            def q_pair(srcT, j, hp, wq_sb, bq_sb, eng):
                """One head pair's Q for query chunk j -> transient tile."""
                qp = lpool.tile([P, TCH], bf16, tag="qp", bufs=1, name="qp")
                q_ps = psA.tile([P, TCH], f32, tag="acc", name="q_ps")
                for kc in range(KC):
                    nc.tensor.matmul(q_ps, wq_sb[:, kc, hp * P:(hp + 1) * P],
                                     srcT[:, kc, j * TCH:(j + 1) * TCH],
                                     start=(kc == 0), stop=(kc == KC - 1))
                evac(qp, q_ps, bq_sb[:, hp:hp + 1], eng)
                return qp

            def attn_pair_j(hp, j, kts, q_src, kT_t, oT_t, fillers):
                """Attention for head pair (2hp, 2hp+1), query chunk j.

                Per k-tile: the two heads' score matmuls target partition
                offsets 0/64 (disjoint PE row groups -> concurrent on HW)
                and land in the two banks of one [P, 2, TCH] f32 PSUM
                tile; a single batched exp covers both heads."""
                hA, hB = 2 * hp, 2 * hp + 1
                qp = q_src(hp, j)
                o_ps = {}
                for hx in (0, 1):
                    o_ps[hx] = psO.tile([AVW, TCH], f32, tag=f"o{hx}", name=f"o_ps{hx}")
                nkt_total = len(kts)
                for done, (kt, bcol, diag) in enumerate(kts):
                    s_ps = psS.tile([P, 2, TCH], f32, tag="s", name="s_ps")
                    pt = lpool.tile([P, 2, TCH], bf16, tag="pt", bufs=3, name="pt")
                    for hx, h in ((0, hA), (1, hB)):
                        off = (h % 2) * DK
                        nc.tensor.matmul(
                            s_ps[:, hx, :],
                            kT_t[off:off + DK, h // 2, kt * P:(kt + 1) * P],
                            qp[off:off + DK, :],
                            start=True, stop=True)
                    bias = zero_sb[:, 0:1] if bcol is None else pbias_sb[:, bcol:bcol + 1]
                    nc.scalar.activation(pt, s_ps, AF.Exp, bias=bias, scale=SCALE)
                    if diag is not None:
                        # all-bf16 SBUF operands -> DVE 4x mode (~200ns)
                        for hx in (0, 1):
                            nc.vector.tensor_mul(pt[:, hx, :], pt[:, hx, :],
                                                 dmask_sb[:, diag, :])
                    for hx, h in ((0, hA), (1, hB)):
                        nc.tensor.matmul(o_ps[hx], vaug[:, kt, h, :],
                                         pt[:, hx, :], start=(done == 0),
                                         stop=(done == nkt_total - 1))
                    if done % 2 == 1:
                        fillers.pop(1)
                # softmax normalization: fast reciprocal + partition broadcast
                for hx, h in ((0, hA), (1, hB)):
                    off = (h % 2) * DK
                    srow = lpool.tile([1, TCH], f32, tag="srow", bufs=1, name="srow")
                    nc.vector.tensor_copy(out=srow, in_=o_ps[hx][DK:AVW, :])
                    rcp = lpool.tile([1, TCH], f32, tag="rcp", bufs=1, name="rcp")
                    nc.vector.reciprocal_approx_fast(out=rcp, in_=srow)
                    rb = lpool.tile([DK, TCH], f32, tag="rb", bufs=1, name="rb")
                    nc.gpsimd.partition_broadcast(rb, rcp, channels=DK)
                    nc.vector.tensor_mul(
                        oT_t[off:off + DK, h // 2, j * TCH:(j + 1) * TCH],
                        o_ps[hx][0:DK, :], rb)

            def out_proj_ft(j, ft, oT_t, wo_sb, bo_sb, eng):
                u_ps = psA.tile([P, TCH], f32, tag="acc", name="u_ps")
                for kc in range(KC):
                    nc.tensor.matmul(u_ps, wo_sb[:, kc, ft * P:(ft + 1) * P],
                                     oT_t[:, kc, j * TCH:(j + 1) * TCH],
                                     start=(kc == 0), stop=(kc == KC - 1))
                # u = wo . oT + h residual, fused on DVE
                nc.vector.scalar_tensor_tensor(
                    out=u_t[:, ft, :], in0=u_ps, scalar=1.0,
                    in1=hbx[:, ft, j * TCH:(j + 1) * TCH],
                    op0=ALU.mult, op1=ALU.add)
                if attn_obias:
                    nc.vector.tensor_scalar(u_t[:, ft, :], u_t[:, ft, :],
                                            bo_sb[:, ft:ft + 1], None, ALU.add)

            def layernorm_t(dsts, g_sb, b_sb):
                """LN over u_t -> dsts(kc)."""
                usq = lpool.tile([P, KC, TCH], bf16, tag="usq", bufs=1, name="usq")
                for kc in range(KC):
                    eng = nc.vector if kc % 2 == 0 else nc.gpsimd
                    eng.tensor_mul(usq[:, kc, :], u_t[:, kc, :], u_t[:, kc, :])
                m_ps = psA.tile([P, TCH], f32, tag="acc", name="m_ps")
                for kc in range(KC):
                    nc.tensor.matmul(m_ps, onesb_sb, u_t[:, kc, :],
                                     start=(kc == 0), stop=(kc == KC - 1))
                q_ps = psA.tile([P, TCH], f32, tag="acc", name="q_ps")
                for kc in range(KC):
                    nc.tensor.matmul(q_ps, onesb_sb, usq[:, kc, :],
                                     start=(kc == 0), stop=(kc == KC - 1))
                t_sb = lpool.tile([P, TCH], f32, tag="lnr", bufs=3, name="t_sb")
                nc.vector.tensor_scalar(t_sb, m_ps, 1.0 / D, None, ALU.mult)
                tt2 = lpool.tile([P, TCH], f32, tag="lnr", bufs=3, name="tt2")
                nc.gpsimd.tensor_mul(tt2, t_sb, t_sb)
                m2 = lpool.tile([P, TCH], f32, tag="lnr", bufs=3, name="m2")
                # m2 = q/D - tt2 in one fused op
                nc.vector.scalar_tensor_tensor(
                    out=m2, in0=q_ps, scalar=1.0 / D, in1=tt2,
                    op0=ALU.mult, op1=ALU.subtract)
                nc.scalar.activation(m2, m2, AF.Ln, bias=eps_sb[:, 0:1])
                r_sb = lpool.tile([P, TCH], f32, tag="lnr", bufs=3, name="r_sb")
                nc.scalar.activation(r_sb, m2, AF.Exp, scale=-0.5, bias=zero_sb[:, 0:1])
                c_sb = lpool.tile([P, TCH], f32, tag="lnc", bufs=2, name="c_sb")
                nc.vector.tensor_mul(c_sb, t_sb, r_sb)
                for kc in range(KC):
                    tmp = lpool.tile([P, TCH], f32, tag="ltmp", bufs=1, name="ltmp")
                    nc.gpsimd.tensor_sub(tmp, u_t[:, kc, :], c_sb)
                    d = dsts(kc)
                    nc.vector.tensor_mul(d, tmp, r_sb)
                    if ln_affine:
                        nc.vector.tensor_scalar(d, d, g_sb[:, kc:kc + 1], b_sb[:, kc:kc + 1],
                                                ALU.mult, ALU.add)

            # ---------------- layer loop ----------------
            pending_fetch = []
            for l in range(NLAYERS):
                last = l == NLAYERS - 1

                # ---- SA projections (ACT idle here -> act evacs) ----
                wk_sa = load_w(w_in["sa_wk"], l, D, "wk", bufs=2)
                wv_sa = load_w(w_in["sa_wv"], l, D, "wv")
                bk_sa = load_b(w_in["sa_bk"], l, "bk")
                bq_sa = load_b(w_in["sa_bq"], l, "bq")
                bv_sa = None
                if v_bias:
                    bv_sa = wpool.tile([1, D], f32, tag="bv", bufs=2, name="bv")
                    nc.sync.dma_start(out=bv_sa, in_=w_in["sa_bv"][l % L:l % L + 1, :])
                wq_sa = load_w(w_in["sa_wq"], l, D, "wq")

                def hbx_chunk(t):
                    return hbx[:, :, t * TCH:(t + 1) * TCH]

                def enc_chunk(t):
                    ec = lpool.tile([P, KC, TCH], bf16, tag="peer", bufs=2,
                                    name="encC")
                    for kc in range(KC):
                        nc.sync.dma_start(out=ec[:, kc, :],
                                          in_=encTb[:, kc, t * TCH:(t + 1) * TCH])
                    return ec

                def peer_chunk(half):
                    pc = lpool.tile([P, KC, TCH], bf16, tag="peer", bufs=2,
                                    name="peerC")
                    if l == 0:
                        for kc in range(KC):
                            nc.sync.dma_start(
                                out=pc[:, kc, :],
                                in_=xTb[:, kc, HALF + half * TCH:HALF + (half + 1) * TCH])
                    else:
                        ccout = pending_fetch[half]
                        nc.sync.dma_start(out=pc, in_=ccout[ts(peer, P), :, :])
                    return pc

                for t in (0, 1):
                    kv_chunk(hbx_chunk(t), t, wk_sa, bk_sa, wv_sa, bv_sa, "act")
                # peer chunk j0 (waits on gather half 0 from the prev layer)
                kv_chunk(peer_chunk(0), 2, wk_sa, bk_sa, wv_sa, bv_sa, "act")

                wo_sa = load_w(w_in["sa_wo"], l, D, "wo")
                bo_sa = load_b(w_in["sa_bo"], l, "bo")
                g1 = load_b(w_in["ln1_g"], l, "g1") if ln_affine else None
                b1l = load_b(w_in["ln1_b"], l, "b1l") if ln_affine else None
                wq_ca = load_w(w_in["ca_wq"], l, D, "wq2")
                bq_ca = load_b(w_in["ca_bq"], l, "bq2")

                # ---- SA attention ----
                # j0 touches only own chunks + peer j0 (gather half 0), so it
                # runs while gather half 1 is still in flight; peer chunk j1
                # projection follows it.
                fill = FillerQueue()
                def sa_q(hp, j):
                    return q_pair(hbx, j, hp, wq_sa, bq_sa, "dve")

                for hp in range(4):
                    attn_pair_j(hp, 0, SA_KTS[0], sa_q, kT, oT_s, fill)
                kv_chunk(peer_chunk(1), 3, wk_sa, bk_sa, wv_sa, bv_sa, "act")
                pending_fetch.clear()
                # fillers for j1: out_proj j0 + LN1 t0
                for ft in range(KC):
                    fill.add(lambda ft=ft: out_proj_ft(0, ft, oT_s, wo_sa, bo_sa, "dve"))
                fill.add(lambda: layernorm_t(
                    lambda kc: x1b[:, kc, 0 * TCH:1 * TCH], g1, b1l))
                for hp in range(4):
                    attn_pair_j(hp, 1, SA_KTS[1], sa_q, kT, oT_s, fill)
                fill.drain()

                # ---- CA K/V from enc (shared kT/vaug) ----
                wk_ca = load_w(w_in["ca_wk"], l, D, "wk")
                bk_ca = load_b(w_in["ca_bk"], l, "bk")
                for t in range(4):
                    k_chunk(enc_chunk(t), t, wk_ca, bk_ca, kT_c, "dve")
                wv_ca = load_w(w_in["ca_wv"], l, D, "wv")
                bv_ca = None
                if v_bias:
                    bv_ca = wpool.tile([1, D], f32, tag="bv", bufs=2, name="bv")
                    nc.sync.dma_start(out=bv_ca, in_=w_in["ca_bv"][l % L:l % L + 1, :])
                for t in range(4):
                    ec = enc_chunk(t)
                    for tl in range(4):
                        tt = t * 4 + tl
                        v_ps = psA.tile([P, D], f32, tag="acc", name="v_ps")
                        nmm = KC + (1 if v_bias else 0)
                        for kc in range(KC):
                            nc.tensor.matmul(v_ps, ec[:, kc, tl * P:(tl + 1) * P],
                                             wv_ca[:, kc, :], start=(kc == 0),
                                             stop=(kc == nmm - 1))
                        if v_bias:
                            nc.tensor.matmul(v_ps, onesb_sb[0:1, :], bv_ca,
                                             start=False, stop=True)
                        nc.vector.tensor_copy(
                            out=vaug[:, tt, :, 0:DK],
                            in_=v_ps.rearrange("p (h d) -> p h d", h=H))

                wo_ca = load_w(w_in["ca_wo"], l, D, "wo")
                bo_ca = load_b(w_in["ca_bo"], l, "bo")
                g2 = load_b(w_in["ln2_g"], l, "g2") if ln_affine else None
                b2l = load_b(w_in["ln2_b"], l, "b2l") if ln_affine else None

                # ---- CA attention ----
                def ca_q(hp, j):
                    return q_pair(x1b, j, hp, wq_ca, bq_ca, "dve")

                fill = FillerQueue()
                # fillers for j0: SA out_proj j1 + LN1 t1
                for ft in range(KC):
                    fill.add(lambda ft=ft: out_proj_ft(1, ft, oT_s, wo_sa, bo_sa, "dve"))
                fill.add(lambda: layernorm_t(
                    lambda kc: x1b[:, kc, 1 * TCH:2 * TCH], g1, b1l))
                for hp in range(4):
                    attn_pair_j(hp, 0, CA_KTS, ca_q, kT_c, oT_c, fill)
                fill.drain()

                w1_sb = load_w(w_in["ff_w1"], l, F, "w1", bufs=1)
                b1_sb = load_b(w_in["ff_b1"], l, "b1")
                w2_sb = wpool.tile([P, NFT, D], bf16, tag="w2", bufs=1, name="w2_sb")
                for kc in range(NFT):
                    nc.sync.dma_start(out=w2_sb[:, kc, :],
                                      in_=w_in["ff_w2b"][l % L, kc * P:(kc + 1) * P, :])
                b2_sb = load_b(w_in["ff_b2"], l, "b2")
                g3 = load_b(w_in["ln3_g"], l, "g3") if ln_affine else None
                b3l = load_b(w_in["ln3_b"], l, "b3l") if ln_affine else None

                def ca_post_j(j):
                    for ft in range(KC):
                        out_proj_ft(j, ft, oT_c, wo_ca, bo_ca, "dve")
                    layernorm_t(lambda kc, j=j: yT[:, kc, j * TCH:(j + 1) * TCH],
                                g2, b2l)

                def ffn1_ft(t, ft):
                    f_ps = psA.tile([P, TCH], f32, tag="acc", name="f_ps")
                    for kc in range(KC):
                        nc.tensor.matmul(f_ps, w1_sb[:, kc, ft * P:(ft + 1) * P],
                                         yT[:, kc, t * TCH:(t + 1) * TCH],
                                         start=(kc == 0), stop=(kc == KC - 1))
                    nc.vector.tensor_scalar(h1[:, ft, :], f_ps,
                                            b1_sb[:, ft:ft + 1], 0.0, ALU.add, ALU.max)

                def ffn1_t(t):
                    for ft in range(NFT):
                        ffn1_ft(t, ft)

                def ffn2_ft(t, ft):
                    g_ps = psA.tile([P, TCH], f32, tag="acc", name="g_ps")
                    for kc in range(NFT):
                        nc.tensor.matmul(g_ps, w2_sb[:, kc, ft * P:(ft + 1) * P],
                                         h1[:, kc, :],
                                         start=(kc == 0), stop=(kc == NFT - 1))
                    # u = ffn2 + x1 residual, fused on DVE
                    nc.vector.scalar_tensor_tensor(
                        out=u_t[:, ft, :], in0=g_ps, scalar=1.0,
                        in1=x1b[:, ft, t * TCH:(t + 1) * TCH],
                        op0=ALU.mult, op1=ALU.add)
                    if ffn_bias:
                        nc.vector.tensor_scalar(u_t[:, ft, :], u_t[:, ft, :],
                                                b2_sb[:, ft:ft + 1], None, ALU.add)

                def ffn2_t(t):
                    for ft in range(KC):
                        ffn2_ft(t, ft)

                def ln3_t(t):
                    if last:
                        # write LN output in place over u_t, then cast-DMA out
                        # (only gpsimd-initiated DMAs can cast bf16 -> f32)
                        layernorm_t(lambda kc: u_t[:, kc, :], g3, b3l)
                        nc.gpsimd.dma_start(out=out_p[:, :, t * TCH:(t + 1) * TCH],
                                            in_=u_t)
                    else:
                        layernorm_t(lambda kc, t=t: hbx[:, kc, t * TCH:(t + 1) * TCH],
                                    g3, b3l)

                def issue_gather(half):
                    """Start the AllGather for own chunk `half`; the peer's
                    copy is fetched into a transient tile next layer."""
                    ccin = dramp.tile([P, KC, TCH], bf16, tag="ccin", bufs=2,
                                      name="ccin")
                    ccout = dramp.tile([2 * P, KC, TCH], bf16, tag="ccout", bufs=2,
                                       name="ccout")
                    nc.sync.dma_start(out=ccin, in_=hbx[:, :, half * TCH:(half + 1) * TCH])
                    nc.gpsimd.collective_compute(
                        "AllGather", ALU.bypass, replica_groups=RG,
                        ins=[ccin.opt()], outs=[ccout.opt()])
                    return ccout

                fill = FillerQueue()
                for ft in range(KC):
                    fill.add(lambda ft=ft: out_proj_ft(0, ft, oT_c, wo_ca, bo_ca, "dve"))
                fill.add(lambda: layernorm_t(
                    lambda kc: yT[:, kc, 0 * TCH:1 * TCH], g2, b2l))
                for ft in range(NFT):
                    fill.add(lambda ft=ft: ffn1_ft(0, ft))
                # chunk-0 FFN2/LN3 + first gather go in as fillers too, so the
                # collective starts as early as possible
                for ft in range(KC):
                    fill.add(lambda ft=ft: ffn2_ft(0, ft))
                fill.add(lambda: ln3_t(0))
                if not last:
                    fill.add(lambda: pending_fetch.append(issue_gather(0)))
                for hp in range(4):
                    attn_pair_j(hp, 1, CA_KTS, ca_q, kT_c, oT_c, fill)
                fill.drain()

                ca_post_j(1)
                ffn1_t(1)
                ffn2_t(1)
                ln3_t(1)
                if not last:
                    pending_fetch.append(issue_gather(1))
                    assert len(pending_fetch) == 2

    nc.finalize()
    return nc


_BUILD_CACHE = {}
LAST_RESULTS = None


def _get_nc(ln_affine, v_bias, ffn_bias=False, attn_obias=False):
    key = (ln_affine, v_bias, ffn_bias, attn_obias, NLAYERS, OPT_ACTSET)
    if key not in _BUILD_CACHE:
        _BUILD_CACHE[key] = build(ln_affine, v_bias, ffn_bias, attn_obias)
    return _BUILD_CACHE[key]


def _to_T(a, dtype):  # [S, D] -> [P, KC, S] feature-major
    return np.ascontiguousarray(a.T.reshape(KC, P, S).transpose(1, 0, 2)).astype(dtype)


def prepare(inputs):
    """Returns (nc, in_maps) for the given full inputs."""
    inp = {k: np.asarray(v) for k, v in inputs.items()}

    ln_affine = not all(
        np.all(inp[f"ln{i}_g"] == 1.0) and np.all(inp[f"ln{i}_b"] == 0.0) for i in (1, 2, 3)
    )
    v_bias = not (np.all(inp["sa_bv"] == 0.0) and np.all(inp["ca_bv"] == 0.0))
    ffn_bias = not np.all(inp["ff_b2"] == 0.0)
    attn_obias = not (np.all(inp["sa_bo"] == 0.0) and np.all(inp["ca_bo"] == 0.0))
    nc = _get_nc(ln_affine, v_bias, ffn_bias, attn_obias)

    ident = np.eye(P, dtype=np.float32)
    pcol = np.arange(P)[:, None]
    qcol = np.arange(TCH)[None, :]
    dmask = np.stack(
        [(qcol >= i * P + pcol) for i in range(4)], axis=1
    ).astype(ml_dtypes.bfloat16)  # [P, 4, TCH]

    shared = {}
    for pre in ("sa", "ca"):
        for nm in ("bq", "bk", "bv", "bo"):
            shared[f"{pre}_{nm}"] = np.ascontiguousarray(inp[f"{pre}_{nm}"], np.float32)
        for nm in ("wq", "wk", "wv"):
            shared[f"{pre}_{nm}"] = inp[f"{pre}_{nm}"].astype(ml_dtypes.bfloat16)
        shared[f"{pre}_wo"] = inp[f"{pre}_wo"].astype(ml_dtypes.bfloat16)
    shared["ff_w1"] = inp["ff_w1"].astype(ml_dtypes.bfloat16)
    shared["ff_b1"] = np.ascontiguousarray(inp["ff_b1"], np.float32)
    shared["ff_w2b"] = inp["ff_w2"].astype(ml_dtypes.bfloat16)
    shared["ff_b2"] = np.ascontiguousarray(inp["ff_b2"], np.float32)
    if ln_affine:
        for i in (1, 2, 3):
            shared[f"ln{i}_g"] = np.ascontiguousarray(inp[f"ln{i}_g"], np.float32)
            shared[f"ln{i}_b"] = np.ascontiguousarray(inp[f"ln{i}_b"], np.float32)
    shared["onesb"] = np.ones((P, P), np.float32).astype(ml_dtypes.bfloat16)
    shared["dmask"] = dmask

    in_maps = []
    for r in range(8):
        b, g = r // 2, r % 2
        mine = [0, 3] if g == 0 else [1, 2]
        theirs = [1, 2] if g == 0 else [0, 3]
        perm = mine + theirs
        xt = np.concatenate([inp["x"][b].T[:, c * TCH:(c + 1) * TCH] for c in perm], axis=1)
        xt = np.ascontiguousarray(xt.reshape(KC, P, S).transpose(1, 0, 2))
        m = dict(shared)
        m["xTb"] = xt.astype(ml_dtypes.bfloat16)
        m["encTb"] = _to_T(np.asarray(inp["enc"][b], np.float32), ml_dtypes.bfloat16)
        pb = np.zeros(12, np.float32)
        # exp-bias columns: j0 kt8-11 -> 0..3 ; j1 kt8-11 -> 4..7 ; j1 kt12-15 -> 8..11
        # Each group of 4 k-tiles lies in one peer global chunk kg; keep iff kg < qg.
        for base, j, kg in ((0, 0, theirs[0]), (4, 1, theirs[0]), (8, 1, theirs[1])):
            pb[base:base + 4] = 0.0 if kg < mine[j] else NEG
        m["pbias"] = np.broadcast_to(pb, (P, 12)).astype(np.float32).copy()
        in_maps.append(m)
    return nc, in_maps


def unshard(results):
    out = np.zeros((B, S, D), np.float32)
    for r in range(8):
        b, g = r // 2, r % 2
        mine = [0, 3] if g == 0 else [1, 2]
        half = results[r]["out"].transpose(1, 0, 2).reshape(D, HALF)
        for j, c in enumerate(mine):
            out[b, c * TCH:(c + 1) * TCH, :] = half[:, j * TCH:(j + 1) * TCH].T
    return out


def kernel(**inputs):
    global LAST_RESULTS
    nc, in_maps = prepare(inputs)

    res = None
    for attempt in range(3):
        try:
            res = run_bass_kernel_spmd(
                nc, in_maps, core_ids=list(range(8)),
                trace=bool(int(os.environ.get("KERNEL_TRACE", "0"))),
            )
            break
        except Exception:
            # first execution after a fresh NEFF compile occasionally flakes
            # on the runtime side; the NEFF cache makes the retry cheap
            if attempt == 2:
                raise
    LAST_RESULTS = res
    return unshard(res.results)


# revision 53
# speedup vs baseline: 1.1972x; 1.0155x over previous
"""Trainium2 Bass kernel: 6-layer decoder (masked self-attn + cross-attn + FFN).

Sharding (8 cores): 4 batch pairs x 2-way sequence-parallel.
Core r: batch r//2, half g=r%2. Global 512-token chunks: g=0 owns [c0,c3],
g=1 owns [c1,c2] (zigzag for causal load balance). The causal structure is
identical across cores (union schedule); per-core differences are data only
(exp-bias columns and diagonal mask constants).

v2 pipeline: token-chunk software pipeline per layer. The residual stream
lives in bf16 (hbx, both halves); the pair exchanges the own half via a
bf16 AllGather that overlaps the next layer's projections. Scores go to
bf16 PSUM in 2-ktile chunks with one batched exp per chunk; heads are
emitted in even/odd pairs whose score matmuls occupy disjoint PE row
groups (concurrent on HW). Softmax normalization uses DVE fast reciprocal
+ gpsimd partition broadcast. out_proj/LN/FFN work is interleaved into the
attention instruction stream as filler so the PE stays busy during exp.
"""

import os
from collections import deque

import numpy as np
import ml_dtypes

import concourse.bass as bass
import concourse.mybir as mybir
import concourse.tile as tile
from concourse import bacc
from concourse.bass import ts
from concourse.bass_utils import run_bass_kernel_spmd

L, B, S, D, H, DK, F = 6, 4, 2048, 512, 8, 64, 2048
P = 128
TCH = 512                 # token chunk = matmul free dim
HALF = S // 2             # tokens owned per core
KC = D // P               # 4 partition chunks of d_model
NFT = F // P              # 16 feature tiles of FFN hidden
NKT = S // P              # 16 k-tiles over full sequence
AVW = DK + 1              # V columns per head + ones column (softmax sum)
CH = 2                    # k-tiles per scores/exp chunk
W2SCL = 16.0              # host pre-scale of fp8 W2 (keeps it out of denormals)
EPS = 1e-5
SCALE = 1.0 / float(np.sqrt(DK))
NEG = -1e9

f32 = mybir.dt.float32
f32r = mybir.dt.float32r
bf16 = mybir.dt.bfloat16
fp8 = mybir.dt.float8e4
AF = mybir.ActivationFunctionType
ALU = mybir.AluOpType

NLAYERS = int(os.environ.get("KERNEL_NLAYERS", str(L)))
OPT_ACTSET = bool(int(os.environ.get("KOPT_ACTSET", "1")))
RG = [[0, 1], [2, 3], [4, 5], [6, 7]]

# Union causal schedule (identical on every core). Local k-tile order:
# 0-3 = my chunk j0, 4-7 = my chunk j1, 8-11 = peer j0, 12-15 = peer j1.
# Entries: (ktile, exp-bias pbias column or None, dmask index or None).
SA_KTS = {
    0: [(0, None, 0), (1, None, 1), (2, None, 2), (3, None, 3),
        (8, 0, None), (9, 0, None), (10, 0, None), (11, 0, None)],
    1: [(0, None, None), (1, None, None), (2, None, None), (3, None, None),
        (4, None, 0), (5, None, 1), (6, None, 2), (7, None, 3),
        (8, 4, None), (9, 4, None), (10, 4, None), (11, 4, None),
        (12, 8, None), (13, 8, None), (14, 8, None), (15, 8, None)],
}
CA_KTS = [(kt, None, None) for kt in range(NKT)]


def _single_act_set():
    # Force every ACT function onto natural_log_exp_and_others (it contains
    # Exp, Ln, Identity and Relu) so the compiled kernel has exactly one
    # ACT_TABLE_LOAD instead of thrashing between per-function sets.
    real = bacc.get_activation_tables

    def patched(arch):
        tabs = real(arch)
        return {name: (fns if name == "natural_log_exp_and_others" else set())
                for name, fns in tabs.items()}

    bacc.get_activation_tables = patched


if OPT_ACTSET:
    _single_act_set()


class FillerQueue:
    """Units of independent work interleaved into attention streams."""

    def __init__(self):
        self.q = deque()

    def add(self, fn):
        self.q.append(fn)

    def pop(self, n=1):
        for _ in range(n):
            if not self.q:
                return
            fn = self.q.popleft()
            if fn is not None:
                fn()

    def drain(self):
        while self.q:
            self.q.popleft()()


def build(ln_affine: bool, v_bias: bool, ffn_bias: bool = False,
          attn_obias: bool = False):
    nc = bacc.Bacc(None, target_bir_lowering=False, num_devices=8)

    xTb = nc.declare_dram_parameter("xTb", [P, KC, S], bf16, isOutput=False)
    encTb = nc.declare_dram_parameter("encTb", [P, KC, S], bf16, isOutput=False)
    w_in = {}
    for pre in ("sa", "ca"):
        for nm in ("wq", "wk", "wv"):
            w_in[f"{pre}_{nm}"] = nc.declare_dram_parameter(f"{pre}_{nm}", [L, D, D], bf16, isOutput=False)
        w_in[f"{pre}_wo"] = nc.declare_dram_parameter(f"{pre}_wo", [L, D, D], bf16, isOutput=False)
        for nm in ("bq", "bk", "bv", "bo"):
            w_in[f"{pre}_{nm}"] = nc.declare_dram_parameter(f"{pre}_{nm}", [L, D], f32, isOutput=False)
    w_in["ff_w1"] = nc.declare_dram_parameter("ff_w1", [L, D, F], bf16, isOutput=False)
    w_in["ff_b1"] = nc.declare_dram_parameter("ff_b1", [L, F], f32, isOutput=False)
    w_in["ff_w2b"] = nc.declare_dram_parameter("ff_w2b", [L, F, D], bf16, isOutput=False)
    w_in["ff_b2"] = nc.declare_dram_parameter("ff_b2", [L, D], f32, isOutput=False)
    if ln_affine:
        for i in (1, 2, 3):
            w_in[f"ln{i}_g"] = nc.declare_dram_parameter(f"ln{i}_g", [L, D], f32, isOutput=False)
            w_in[f"ln{i}_b"] = nc.declare_dram_parameter(f"ln{i}_b", [L, D], f32, isOutput=False)
    onesb_in = nc.declare_dram_parameter("onesb", [P, P], bf16, isOutput=False)
    dmask_in = nc.declare_dram_parameter("dmask", [P, 4, TCH], bf16, isOutput=False)
    pbias_in = nc.declare_dram_parameter("pbias", [P, 12], f32, isOutput=False)
    out_p = nc.declare_dram_parameter("out", [P, KC, HALF], f32, isOutput=True)

    with tile.TileContext(nc, num_cores=8) as tc:
        import contextlib

        gctx = contextlib.ExitStack()
        with gctx:
            persist = gctx.enter_context(tc.tile_pool(name="persist", bufs=1))
            wpool = gctx.enter_context(tc.tile_pool(name="wpool", bufs=1))
            lpool = gctx.enter_context(tc.tile_pool(name="lpool", bufs=1))
            psS = gctx.enter_context(tc.tile_pool(name="psS", bufs=2, space="PSUM"))
            psO = gctx.enter_context(tc.tile_pool(name="psO", bufs=1, space="PSUM"))
            psA = gctx.enter_context(tc.tile_pool(name="psA", bufs=2, space="PSUM"))
            dramp = gctx.enter_context(tc.tile_pool(name="dramp", bufs=2, space="DRAM"))

            # ---- persistent state (SBUF) ----
            hbx = persist.tile([P, KC, HALF], bf16, name="hbx")  # own residual
            kT = persist.tile([P, KC, S], bf16, name="kT")       # shared SA/CA K^T
            kT_c = kT
            vaug = persist.tile([P, NKT, H, AVW], bf16, name="vaug")  # shared aug-V
            oT_s = persist.tile([P, KC, HALF], bf16, name="oT_s")
            oT_c = persist.tile([P, KC, HALF], bf16, name="oT_c")
            x1b = persist.tile([P, KC, HALF], bf16, name="x1b")
            yT = persist.tile([P, KC, HALF], bf16, name="yT")
            h1 = persist.tile([P, NFT, TCH], bf16, name="h1")
            u_t = persist.tile([P, KC, TCH], bf16, name="u_t")   # psum-evac target

            onesb_sb = persist.tile([P, P], bf16, name="onesb_sb")
            dmask_sb = persist.tile([P, 4, TCH], bf16, name="dmask_sb")
            pbias_sb = persist.tile([P, 12], f32, name="pbias_sb")
            zero_sb = persist.tile([P, 1], f32, name="zero_sb")
            eps_sb = persist.tile([P, 1], f32, name="eps_sb")
            nc.vector.memset(zero_sb, 0.0)
            nc.vector.memset(eps_sb, EPS)
            # ones columns of the augmented-V layout, set once (V writes
            # never touch them, across all layers and both attentions)
            nc.vector.memset(vaug[:, :, :, DK:DK + 1], 1.0)

            for kc in range(KC):
                nc.sync.dma_start(out=hbx[:, kc, :], in_=xTb[:, kc, 0:HALF])
            nc.sync.dma_start(out=onesb_sb, in_=onesb_in[:, :])
            nc.sync.dma_start(out=dmask_sb, in_=dmask_in[:, :, :])
            nc.sync.dma_start(out=pbias_sb, in_=pbias_in[:, :])

            pid = nc.sync.partition_id()
            peer = (pid + 1) % 2

            def load_w(dram_t, l, cols, tag, bufs=2, dt=bf16):
                n = dram_t.shape[1] // P
                l = l % L
                t = wpool.tile([P, n, cols], dt, tag=tag, bufs=bufs, name=tag)
                for kc in range(n):
                    nc.sync.dma_start(out=t[:, kc, :], in_=dram_t[l, kc * P:(kc + 1) * P, :])
                return t

            def load_b(dram_t, l, tag):
                n = dram_t.shape[1] // P
                l = l % L
                t = wpool.tile([P, n], f32, tag=tag, bufs=2, name=tag)
                nc.sync.dma_start(out=t, in_=dram_t[l].rearrange("(c p) -> p c", p=P))
                return t

            def evac(dst, src_ps, bias_col, eng):
                if eng == "act":
                    nc.scalar.activation(dst, src_ps, AF.Identity, bias=bias_col)
                else:
                    nc.vector.tensor_scalar(dst, src_ps, bias_col, None, ALU.add)

            # ---------------- building blocks ----------------

            def k_chunk(src, t, wk_sb, bk_sb, kT_t, eng):
                for ft in range(KC):
                    k_ps = psA.tile([P, TCH], f32, tag="acc", name="k_ps")
                    for kc in range(KC):
                        nc.tensor.matmul(k_ps, wk_sb[:, kc, ft * P:(ft + 1) * P],
                                         src[:, kc, :], start=(kc == 0), stop=(kc == KC - 1))
                    evac(kT_t[:, ft, t * TCH:(t + 1) * TCH], k_ps, bk_sb[:, ft:ft + 1], eng)

            def kv_chunk(src, t, wk_sb, bk_sb, wv_sb, bv_sb, eng):
                """K^T + augmented V for one 512-token chunk t (bf16)."""
                k_chunk(src, t, wk_sb, bk_sb, kT, eng)
                for tl in range(4):
                    tt = t * 4 + tl
                    v_ps = psA.tile([P, D], f32, tag="acc", name="v_ps")
                    nmm = KC + (1 if v_bias else 0)
                    for kc in range(KC):
                        nc.tensor.matmul(v_ps, src[:, kc, tl * P:(tl + 1) * P],
                                         wv_sb[:, kc, :], start=(kc == 0),
                                         stop=(kc == nmm - 1))
                    if v_bias:
                        nc.tensor.matmul(v_ps, onesb_sb[0:1, :], bv_sb, start=False, stop=True)
                    # one strided copy drops all 8 heads into the aug layout
                    nc.vector.tensor_copy(
                        out=vaug[:, tt, :, 0:DK],
                        in_=v_ps.rearrange("p (h d) -> p h d", h=H))

            def q_pair(srcT, j, hp, wq_sb, bq_sb, eng):
                """One head pair's Q for query chunk j -> transient tile."""
                qp = lpool.tile([P, TCH], bf16, tag="qp", bufs=1, name="qp")
                q_ps = psA.tile([P, TCH], f32, tag="acc", name="q_ps")
                for kc in range(KC):
                    nc.tensor.matmul(q_ps, wq_sb[:, kc, hp * P:(hp + 1) * P],
                                     srcT[:, kc, j * TCH:(j + 1) * TCH],
                                     start=(kc == 0), stop=(kc == KC - 1))
                evac(qp, q_ps, bq_sb[:, hp:hp + 1], eng)
                return qp

            def attn_pair_j(hp, j, kts, q_src, kT_t, oT_t, fillers):
                """Attention for head pair (2hp, 2hp+1), query chunk j.

                Per k-tile: the two heads' score matmuls target partition
                offsets 0/64 (disjoint PE row groups -> concurrent on HW)
                and land in the two banks of one [P, 2, TCH] f32 PSUM
                tile; a single batched exp covers both heads."""
                hA, hB = 2 * hp, 2 * hp + 1
                qp = q_src(hp, j)
                o_ps = {}
                for hx in (0, 1):
                    o_ps[hx] = psO.tile([AVW, TCH], f32, tag=f"o{hx}", name=f"o_ps{hx}")
                nkt_total = len(kts)
                for done, (kt, bcol, diag) in enumerate(kts):
                    s_ps = psS.tile([P, 2, TCH], f32, tag="s", name="s_ps")
                    pt = lpool.tile([P, 2, TCH], bf16, tag="pt", bufs=3, name="pt")
                    for hx, h in ((0, hA), (1, hB)):
                        off = (h % 2) * DK
                        nc.tensor.matmul(
                            s_ps[:, hx, :],
                            kT_t[off:off + DK, h // 2, kt * P:(kt + 1) * P],
                            qp[off:off + DK, :],
                            start=True, stop=True)
                    bias = zero_sb[:, 0:1] if bcol is None else pbias_sb[:, bcol:bcol + 1]
                    nc.scalar.activation(pt, s_ps, AF.Exp, bias=bias, scale=SCALE)
                    if diag is not None:
                        # all-bf16 SBUF operands -> DVE 4x mode (~200ns)
                        for hx in (0, 1):
                            nc.vector.tensor_mul(pt[:, hx, :], pt[:, hx, :],
                                                 dmask_sb[:, diag, :])
                    for hx, h in ((0, hA), (1, hB)):
                        nc.tensor.matmul(o_ps[hx], vaug[:, kt, h, :],
                                         pt[:, hx, :], start=(done == 0),
                                         stop=(done == nkt_total - 1))
                    if done % 2 == 1:
                        fillers.pop(1)
                # softmax normalization: fast reciprocal + partition broadcast
                for hx, h in ((0, hA), (1, hB)):
                    off = (h % 2) * DK
                    srow = lpool.tile([1, TCH], f32, tag="srow", bufs=1, name="srow")
                    nc.vector.tensor_copy(out=srow, in_=o_ps[hx][DK:AVW, :])
                    rcp = lpool.tile([1, TCH], f32, tag="rcp", bufs=1, name="rcp")
                    nc.vector.reciprocal_approx_fast(out=rcp, in_=srow)
                    rb = lpool.tile([DK, TCH], f32, tag="rb", bufs=1, name="rb")
                    nc.gpsimd.partition_broadcast(rb, rcp, channels=DK)
                    nc.vector.tensor_mul(
                        oT_t[off:off + DK, h // 2, j * TCH:(j + 1) * TCH],
                        o_ps[hx][0:DK, :], rb)

            def out_proj_ft(j, ft, oT_t, wo_sb, bo_sb, eng):
                u_ps = psA.tile([P, TCH], f32, tag="acc", name="u_ps")
                for kc in range(KC):
                    nc.tensor.matmul(u_ps, wo_sb[:, kc, ft * P:(ft + 1) * P],
                                     oT_t[:, kc, j * TCH:(j + 1) * TCH],
                                     start=(kc == 0), stop=(kc == KC - 1))
                # u = wo . oT + h residual, fused on DVE
                nc.vector.scalar_tensor_tensor(
                    out=u_t[:, ft, :], in0=u_ps, scalar=1.0,
                    in1=hbx[:, ft, j * TCH:(j + 1) * TCH],
                    op0=ALU.mult, op1=ALU.add)
                if attn_obias:
                    nc.vector.tensor_scalar(u_t[:, ft, :], u_t[:, ft, :],
                                            bo_sb[:, ft:ft + 1], None, ALU.add)

            def layernorm_t(dsts, g_sb, b_sb):
                """LN over u_t -> dsts(kc)."""
                usq = lpool.tile([P, KC, TCH], bf16, tag="usq", bufs=1, name="usq")
                for kc in range(KC):
                    eng = nc.vector if kc % 2 == 0 else nc.gpsimd
                    eng.tensor_mul(usq[:, kc, :], u_t[:, kc, :], u_t[:, kc, :])
                m_ps = psA.tile([P, TCH], f32, tag="acc", name="m_ps")
                for kc in range(KC):
                    nc.tensor.matmul(m_ps, onesb_sb, u_t[:, kc, :],
                                     start=(kc == 0), stop=(kc == KC - 1))
                q_ps = psA.tile([P, TCH], f32, tag="acc", name="q_ps")
                for kc in range(KC):
                    nc.tensor.matmul(q_ps, onesb_sb, usq[:, kc, :],
                                     start=(kc == 0), stop=(kc == KC - 1))
                t_sb = lpool.tile([P, TCH], f32, tag="lnr", bufs=3, name="t_sb")
                nc.vector.tensor_scalar(t_sb, m_ps, 1.0 / D, None, ALU.mult)
                tt2 = lpool.tile([P, TCH], f32, tag="lnr", bufs=3, name="tt2")
                nc.gpsimd.tensor_mul(tt2, t_sb, t_sb)
                m2 = lpool.tile([P, TCH], f32, tag="lnr", bufs=3, name="m2")
                # m2 = q/D - tt2 in one fused op
                nc.vector.scalar_tensor_tensor(
                    out=m2, in0=q_ps, scalar=1.0 / D, in1=tt2,
                    op0=ALU.mult, op1=ALU.subtract)
                nc.scalar.activation(m2, m2, AF.Ln, bias=eps_sb[:, 0:1])
                r_sb = lpool.tile([P, TCH], f32, tag="lnr", bufs=3, name="r_sb")
                nc.scalar.activation(r_sb, m2, AF.Exp, scale=-0.5, bias=zero_sb[:, 0:1])
                c_sb = lpool.tile([P, TCH], f32, tag="lnc", bufs=2, name="c_sb")
                nc.vector.tensor_mul(c_sb, t_sb, r_sb)
                for kc in range(KC):
                    tmp = lpool.tile([P, TCH], f32, tag="ltmp", bufs=1, name="ltmp")
                    nc.gpsimd.tensor_sub(tmp, u_t[:, kc, :], c_sb)
                    d = dsts(kc)
                    nc.vector.tensor_mul(d, tmp, r_sb)
                    if ln_affine:
                        nc.vector.tensor_scalar(d, d, g_sb[:, kc:kc + 1], b_sb[:, kc:kc + 1],
                                                ALU.mult, ALU.add)

            # ---------------- layer loop ----------------
            pending_fetch = []
            for l in range(NLAYERS):
                last = l == NLAYERS - 1

                # ---- SA projections (ACT idle here -> act evacs) ----
                wk_sa = load_w(w_in["sa_wk"], l, D, "wk", bufs=2)
                wv_sa = load_w(w_in["sa_wv"], l, D, "wv")
                bk_sa = load_b(w_in["sa_bk"], l, "bk")
                bq_sa = load_b(w_in["sa_bq"], l, "bq")
                bv_sa = None
                if v_bias:
                    bv_sa = wpool.tile([1, D], f32, tag="bv", bufs=2, name="bv")
                    nc.sync.dma_start(out=bv_sa, in_=w_in["sa_bv"][l % L:l % L + 1, :])
                wq_sa = load_w(w_in["sa_wq"], l, D, "wq")

                def hbx_chunk(t):
                    return hbx[:, :, t * TCH:(t + 1) * TCH]

                def enc_chunk(t):
                    ec = lpool.tile([P, KC, TCH], bf16, tag="peer", bufs=2,
                                    name="encC")
                    for kc in range(KC):
                        nc.sync.dma_start(out=ec[:, kc, :],
                                          in_=encTb[:, kc, t * TCH:(t + 1) * TCH])
                    return ec

                def peer_chunk(half):
                    pc = lpool.tile([P, KC, TCH], bf16, tag="peer", bufs=2,
                                    name="peerC")
                    if l == 0:
                        for kc in range(KC):
                            nc.sync.dma_start(
                                out=pc[:, kc, :],
                                in_=xTb[:, kc, HALF + half * TCH:HALF + (half + 1) * TCH])
                    else:
                        ccout = pending_fetch[half]
                        nc.sync.dma_start(out=pc, in_=ccout[ts(peer, P), :, :])
                    return pc

                for t in (0, 1):
                    kv_chunk(hbx_chunk(t), t, wk_sa, bk_sa, wv_sa, bv_sa, "act")
                # peer chunk j0 (waits on gather half 0 from the prev layer)
                kv_chunk(peer_chunk(0), 2, wk_sa, bk_sa, wv_sa, bv_sa, "act")

                wo_sa = load_w(w_in["sa_wo"], l, D, "wo")
                bo_sa = load_b(w_in["sa_bo"], l, "bo")
                g1 = load_b(w_in["ln1_g"], l, "g1") if ln_affine else None
                b1l = load_b(w_in["ln1_b"], l, "b1l") if ln_affine else None
                wq_ca = load_w(w_in["ca_wq"], l, D, "wq2")
                bq_ca = load_b(w_in["ca_bq"], l, "bq2")

                # ---- SA attention ----
                # j0 touches only own chunks + peer j0 (gather half 0), so it
                # runs while gather half 1 is still in flight; peer chunk j1
                # projection follows it.
                fill = FillerQueue()
                def sa_q(hp, j):
                    return q_pair(hbx, j, hp, wq_sa, bq_sa, "dve")

                for hp in range(4):
                    attn_pair_j(hp, 0, SA_KTS[0], sa_q, kT, oT_s, fill)
                kv_chunk(peer_chunk(1), 3, wk_sa, bk_sa, wv_sa, bv_sa, "act")
                pending_fetch.clear()
                # fillers for j1: out_proj j0 + LN1 t0
                for ft in range(KC):
                    fill.add(lambda ft=ft: out_proj_ft(0, ft, oT_s, wo_sa, bo_sa, "dve"))
                fill.add(lambda: layernorm_t(
                    lambda kc: x1b[:, kc, 0 * TCH:1 * TCH], g1, b1l))
                for hp in range(4):
                    attn_pair_j(hp, 1, SA_KTS[1], sa_q, kT, oT_s, fill)
                fill.drain()

                # ---- CA K/V from enc (shared kT/vaug) ----
                wk_ca = load_w(w_in["ca_wk"], l, D, "wk")
                bk_ca = load_b(w_in["ca_bk"], l, "bk")
                for t in range(4):
                    k_chunk(enc_chunk(t), t, wk_ca, bk_ca, kT_c, "dve")
                wv_ca = load_w(w_in["ca_wv"], l, D, "wv")
                bv_ca = None
                if v_bias:
                    bv_ca = wpool.tile([1, D], f32, tag="bv", bufs=2, name="bv")
                    nc.sync.dma_start(out=bv_ca, in_=w_in["ca_bv"][l % L:l % L + 1, :])
                for t in range(4):
                    ec = enc_chunk(t)
                    for tl in range(4):
                        tt = t * 4 + tl
                        v_ps = psA.tile([P, D], f32, tag="acc", name="v_ps")
                        nmm = KC + (1 if v_bias else 0)
                        for kc in range(KC):
                            nc.tensor.matmul(v_ps, ec[:, kc, tl * P:(tl + 1) * P],
                                             wv_ca[:, kc, :], start=(kc == 0),
                                             stop=(kc == nmm - 1))
                        if v_bias:
                            nc.tensor.matmul(v_ps, onesb_sb[0:1, :], bv_ca,
                                             start=False, stop=True)
                        nc.vector.tensor_copy(
                            out=vaug[:, tt, :, 0:DK],
                            in_=v_ps.rearrange("p (h d) -> p h d", h=H))

                wo_ca = load_w(w_in["ca_wo"], l, D, "wo")
                bo_ca = load_b(w_in["ca_bo"], l, "bo")
                g2 = load_b(w_in["ln2_g"], l, "g2") if ln_affine else None
                b2l = load_b(w_in["ln2_b"], l, "b2l") if ln_affine else None

                # ---- CA attention ----
                def ca_q(hp, j):
                    return q_pair(x1b, j, hp, wq_ca, bq_ca, "dve")

                fill = FillerQueue()
                # fillers for j0: SA out_proj j1 + LN1 t1
                for ft in range(KC):
                    fill.add(lambda ft=ft: out_proj_ft(1, ft, oT_s, wo_sa, bo_sa, "dve"))
                fill.add(lambda: layernorm_t(
                    lambda kc: x1b[:, kc, 1 * TCH:2 * TCH], g1, b1l))
                for hp in range(4):
                    attn_pair_j(hp, 0, CA_KTS, ca_q, kT_c, oT_c, fill)
                fill.drain()

                w1_sb = load_w(w_in["ff_w1"], l, F, "w1", bufs=1)
                b1_sb = load_b(w_in["ff_b1"], l, "b1")
                w2_sb = wpool.tile([P, NFT, D], bf16, tag="w2", bufs=1, name="w2_sb")
                for kc in range(NFT):
                    nc.sync.dma_start(out=w2_sb[:, kc, :],
                                      in_=w_in["ff_w2b"][l % L, kc * P:(kc + 1) * P, :])
                b2_sb = load_b(w_in["ff_b2"], l, "b2")
                g3 = load_b(w_in["ln3_g"], l, "g3") if ln_affine else None
                b3l = load_b(w_in["ln3_b"], l, "b3l") if ln_affine else None

                def ca_post_j(j):
                    for ft in range(KC):
                        out_proj_ft(j, ft, oT_c, wo_ca, bo_ca, "dve")
                    layernorm_t(lambda kc, j=j: yT[:, kc, j * TCH:(j + 1) * TCH],
                                g2, b2l)

                def ffn1_ft(t, ft):
                    f_ps = psA.tile([P, TCH], f32, tag="acc", name="f_ps")
                    for kc in range(KC):
                        nc.tensor.matmul(f_ps, w1_sb[:, kc, ft * P:(ft + 1) * P],
                                         yT[:, kc, t * TCH:(t + 1) * TCH],
                                         start=(kc == 0), stop=(kc == KC - 1))
                    nc.vector.tensor_scalar(h1[:, ft, :], f_ps,
                                            b1_sb[:, ft:ft + 1], 0.0, ALU.add, ALU.max)

                def ffn1_t(t):
                    for ft in range(NFT):
                        ffn1_ft(t, ft)

                def ffn2_ft(t, ft):
                    g_ps = psA.tile([P, TCH], f32, tag="acc", name="g_ps")
                    for kc in range(NFT):
                        nc.tensor.matmul(g_ps, w2_sb[:, kc, ft * P:(ft + 1) * P],
                                         h1[:, kc, :],
                                         start=(kc == 0), stop=(kc == NFT - 1))
                    # u = ffn2 + x1 residual, fused on DVE
                    nc.vector.scalar_tensor_tensor(
                        out=u_t[:, ft, :], in0=g_ps, scalar=1.0,
                        in1=x1b[:, ft, t * TCH:(t + 1) * TCH],
                        op0=ALU.mult, op1=ALU.add)
                    if ffn_bias:
                        nc.vector.tensor_scalar(u_t[:, ft, :], u_t[:, ft, :],
                                                b2_sb[:, ft:ft + 1], None, ALU.add)

                def ffn2_t(t):
                    for ft in range(KC):
                        ffn2_ft(t, ft)

                def ln3_t(t):
                    if last:
                        # write LN output in place over u_t, then cast-DMA out
                        # (only gpsimd-initiated DMAs can cast bf16 -> f32)
                        layernorm_t(lambda kc: u_t[:, kc, :], g3, b3l)
                        nc.gpsimd.dma_start(out=out_p[:, :, t * TCH:(t + 1) * TCH],
                                            in_=u_t)
                    else:
                        layernorm_t(lambda kc, t=t: hbx[:, kc, t * TCH:(t + 1) * TCH],
                                    g3, b3l)

                def issue_gather(half):
                    """Start the AllGather for own chunk `half`; the peer's
                    copy is fetched into a transient tile next layer."""
                    ccin = dramp.tile([P, KC, TCH], bf16, tag="ccin", bufs=2,
                                      name="ccin")
                    ccout = dramp.tile([2 * P, KC, TCH], bf16, tag="ccout", bufs=2,
                                       name="ccout")
                    nc.sync.dma_start(out=ccin, in_=hbx[:, :, half * TCH:(half + 1) * TCH])
                    nc.gpsimd.collective_compute(
                        "AllGather", ALU.bypass, replica_groups=RG,
                        ins=[ccin.opt()], outs=[ccout.opt()])
                    return ccout

                fill = FillerQueue()
                for ft in range(KC):
                    fill.add(lambda ft=ft: out_proj_ft(0, ft, oT_c, wo_ca, bo_ca, "dve"))
                fill.add(lambda: layernorm_t(
                    lambda kc: yT[:, kc, 0 * TCH:1 * TCH], g2, b2l))
                for ft in range(NFT):
                    fill.add(lambda ft=ft: ffn1_ft(0, ft))
                # chunk-0 FFN2/LN3 + first gather go in as fillers too, so the
                # collective starts as early as possible
                for ft in range(KC):
                    fill.add(lambda ft=ft: ffn2_ft(0, ft))
                fill.add(lambda: ln3_t(0))
                if not last:
                    fill.add(lambda: pending_fetch.append(issue_gather(0)))
                for hp in range(4):
                    attn_pair_j(hp, 1, CA_KTS, ca_q, kT_c, oT_c, fill)
                fill.drain()

                ca_post_j(1)
                ffn1_t(1)
                ffn2_t(1)
                ln3_t(1)
                if not last:
                    pending_fetch.append(issue_gather(1))
                    assert len(pending_fetch) == 2

    nc.finalize()
    return nc


_BUILD_CACHE = {}
LAST_RESULTS = None


def _get_nc(ln_affine, v_bias, ffn_bias=False, attn_obias=False):
    key = (ln_affine, v_bias, ffn_bias, attn_obias, NLAYERS, OPT_ACTSET)
    if key not in _BUILD_CACHE:
        _BUILD_CACHE[key] = build(ln_affine, v_bias, ffn_bias, attn_obias)
    return _BUILD_CACHE[key]


def _to_T(a, dtype):  # [S, D] -> [P, KC, S] feature-major
    return np.ascontiguousarray(a.T.reshape(KC, P, S).transpose(1, 0, 2)).astype(dtype)


def prepare(inputs):
    """Returns (nc, in_maps) for the given full inputs."""
    inp = {k: np.asarray(v) for k, v in inputs.items()}

    ln_affine = not all(
        np.all(inp[f"ln{i}_g"] == 1.0) and np.all(inp[f"ln{i}_b"] == 0.0) for i in (1, 2, 3)
    )
    v_bias = not (np.all(inp["sa_bv"] == 0.0) and np.all(inp["ca_bv"] == 0.0))
    ffn_bias = not np.all(inp["ff_b2"] == 0.0)
    attn_obias = not (np.all(inp["sa_bo"] == 0.0) and np.all(inp["ca_bo"] == 0.0))
    nc = _get_nc(ln_affine, v_bias, ffn_bias, attn_obias)

    ident = np.eye(P, dtype=np.float32)
    pcol = np.arange(P)[:, None]
    qcol = np.arange(TCH)[None, :]
    dmask = np.stack(
        [(qcol >= i * P + pcol) for i in range(4)], axis=1
    ).astype(ml_dtypes.bfloat16)  # [P, 4, TCH]

    shared = {}
    for pre in ("sa", "ca"):
        for nm in ("bq", "bk", "bv", "bo"):
            shared[f"{pre}_{nm}"] = np.ascontiguousarray(inp[f"{pre}_{nm}"], np.float32)
        for nm in ("wq", "wk", "wv"):
            shared[f"{pre}_{nm}"] = inp[f"{pre}_{nm}"].astype(ml_dtypes.bfloat16)
        shared[f"{pre}_wo"] = inp[f"{pre}_wo"].astype(ml_dtypes.bfloat16)
    shared["ff_w1"] = inp["ff_w1"].astype(ml_dtypes.bfloat16)
    shared["ff_b1"] = np.ascontiguousarray(inp["ff_b1"], np.float32)
    shared["ff_w2b"] = inp["ff_w2"].astype(ml_dtypes.bfloat16)
    shared["ff_b2"] = np.ascontiguousarray(inp["ff_b2"], np.float32)
    if ln_affine:
        for i in (1, 2, 3):
            shared[f"ln{i}_g"] = np.ascontiguousarray(inp[f"ln{i}_g"], np.float32)
            shared[f"ln{i}_b"] = np.ascontiguousarray(inp[f"ln{i}_b"], np.float32)
    shared["onesb"] = np.ones((P, P), np.float32).astype(ml_dtypes.bfloat16)
    shared["dmask"] = dmask

    in_maps = []
    for r in range(8):
        b, g = r // 2, r % 2
        mine = [0, 3] if g == 0 else [1, 2]
        theirs = [1, 2] if g == 0 else [0, 3]
        perm = mine + theirs
        xt = np.concatenate([inp["x"][b].T[:, c * TCH:(c + 1) * TCH] for c in perm], axis=1)
        xt = np.ascontiguousarray(xt.reshape(KC, P, S).transpose(1, 0, 2))
        m = dict(shared)
        m["xTb"] = xt.astype(ml_dtypes.bfloat16)
        m["encTb"] = _to_T(np.asarray(inp["enc"][b], np.float32), ml_dtypes.bfloat16)
        pb = np.zeros(12, np.float32)
        # exp-bias columns: j0 kt8-11 -> 0..3 ; j1 kt8-11 -> 4..7 ; j1 kt12-15 -> 8..11
        # Each group of 4 k-tiles lies in one peer global chunk kg; keep iff kg < qg.
        for base, j, kg in ((0, 0, theirs[0]), (4, 1, theirs[0]), (8, 1, theirs[1])):
            pb[base:base + 4] = 0.0 if kg < mine[j] else NEG
        m["pbias"] = np.broadcast_to(pb, (P, 12)).astype(np.float32).copy()
        in_maps.append(m)
    return nc, in_maps


def unshard(results):
    out = np.zeros((B, S, D), np.float32)
    for r in range(8):
        b, g = r // 2, r % 2
        mine = [0, 3] if g == 0 else [1, 2]
        half = results[r]["out"].transpose(1, 0, 2).reshape(D, HALF)
        for j, c in enumerate(mine):
            out[b, c * TCH:(c + 1) * TCH, :] = half[:, j * TCH:(j + 1) * TCH].T
    return out


def kernel(**inputs):
    global LAST_RESULTS
    nc, in_maps = prepare(inputs)

    res = None
    for attempt in range(3):
        try:
            res = run_bass_kernel_spmd(
                nc, in_maps, core_ids=list(range(8)),
                trace=bool(int(os.environ.get("KERNEL_TRACE", "0"))),
            )
            break
        except Exception:
            # first execution after a fresh NEFF compile occasionally flakes
            # on the runtime side; the NEFF cache makes the retry cheap
            if attempt == 2:
                raise
    LAST_RESULTS = res
    return unshard(res.results)


# revision 54
# speedup vs baseline: 1.2883x; 1.0761x over previous
"""Trainium2 Bass kernel: 6-layer decoder (masked self-attn + cross-attn + FFN).

Sharding (8 cores): 4 batch pairs x 2-way sequence-parallel.
Core r: batch r//2, half g=r%2. Global 512-token chunks: g=0 owns [c0,c3],
g=1 owns [c1,c2] (zigzag for causal load balance). The causal structure is
identical across cores (union schedule); per-core differences are data only
(exp-bias columns and diagonal mask constants).

v2 pipeline: token-chunk software pipeline per layer. The residual stream
lives in bf16 (hbx, both halves); the pair exchanges the own half via a
bf16 AllGather that overlaps the next layer's projections. Scores go to
bf16 PSUM in 2-ktile chunks with one batched exp per chunk; heads are
emitted in even/odd pairs whose score matmuls occupy disjoint PE row
groups (concurrent on HW). Softmax normalization uses DVE fast reciprocal
+ gpsimd partition broadcast. out_proj/LN/FFN work is interleaved into the
attention instruction stream as filler so the PE stays busy during exp.
"""

import os
from collections import deque

import numpy as np
import ml_dtypes

import concourse.bass as bass
import concourse.mybir as mybir
import concourse.tile as tile
from concourse import bacc
from concourse.bass import ts
from concourse.bass_utils import run_bass_kernel_spmd

L, B, S, D, H, DK, F = 6, 4, 2048, 512, 8, 64, 2048
P = 128
TCH = 512                 # token chunk = matmul free dim
HALF = S // 2             # tokens owned per core
KC = D // P               # 4 partition chunks of d_model
NFT = F // P              # 16 feature tiles of FFN hidden
NKT = S // P              # 16 k-tiles over full sequence
AVW = DK + 1              # V columns per head + ones column (softmax sum)
CH = 2                    # k-tiles per scores/exp chunk
W2SCL = 16.0              # host pre-scale of fp8 W2 (keeps it out of denormals)
EPS = 1e-5
SCALE = 1.0 / float(np.sqrt(DK))
NEG = -1e9

f32 = mybir.dt.float32
f32r = mybir.dt.float32r
bf16 = mybir.dt.bfloat16
fp8 = mybir.dt.float8e4
AF = mybir.ActivationFunctionType
ALU = mybir.AluOpType

NLAYERS = int(os.environ.get("KERNEL_NLAYERS", str(L)))
OPT_ACTSET = bool(int(os.environ.get("KOPT_ACTSET", "1")))
RG = [[0, 1], [2, 3], [4, 5], [6, 7]]

# Union causal schedule (identical on every core). Local k-tile order:
# 0-3 = my chunk j0, 4-7 = my chunk j1, 8-11 = peer j0, 12-15 = peer j1.
# Entries: (ktile, exp-bias pbias column or None, dmask index or None).
SA_KTS = {
    0: [(0, None, 0), (1, None, 1), (2, None, 2), (3, None, 3),
        (8, 0, None), (9, 0, None), (10, 0, None), (11, 0, None)],
    1: [(0, None, None), (1, None, None), (2, None, None), (3, None, None),
        (4, None, 0), (5, None, 1), (6, None, 2), (7, None, 3),
        (8, 4, None), (9, 4, None), (10, 4, None), (11, 4, None),
        (12, 8, None), (13, 8, None), (14, 8, None), (15, 8, None)],
}
CA_KTS = [(kt, None, None) for kt in range(NKT)]


def _single_act_set():
    # Force every ACT function onto natural_log_exp_and_others (it contains
    # Exp, Ln, Identity and Relu) so the compiled kernel has exactly one
    # ACT_TABLE_LOAD instead of thrashing between per-function sets.
    real = bacc.get_activation_tables

    def patched(arch):
        tabs = real(arch)
        return {name: (fns if name == "natural_log_exp_and_others" else set())
                for name, fns in tabs.items()}

    bacc.get_activation_tables = patched


if OPT_ACTSET:
    _single_act_set()


class FillerQueue:
    """Units of independent work interleaved into attention streams."""

    def __init__(self):
        self.q = deque()

    def add(self, fn):
        self.q.append(fn)

    def pop(self, n=1):
        for _ in range(n):
            if not self.q:
                return
            fn = self.q.popleft()
            if fn is not None:
                fn()

    def drain(self):
        while self.q:
            self.q.popleft()()


def build(ln_affine: bool, v_bias: bool, ffn_bias: bool = False,
          attn_obias: bool = False):
    nc = bacc.Bacc(None, target_bir_lowering=False, num_devices=8)

    xTb = nc.declare_dram_parameter("xTb", [P, KC, S], bf16, isOutput=False)
    encTb = nc.declare_dram_parameter("encTb", [P, KC, S], bf16, isOutput=False)
    w_in = {}
    for pre in ("sa", "ca"):
        for nm in ("wq", "wk", "wv"):
            w_in[f"{pre}_{nm}"] = nc.declare_dram_parameter(f"{pre}_{nm}", [L, D, D], bf16, isOutput=False)
        w_in[f"{pre}_wo"] = nc.declare_dram_parameter(f"{pre}_wo", [L, D, D], bf16, isOutput=False)
        for nm in ("bq", "bk", "bv", "bo"):
            w_in[f"{pre}_{nm}"] = nc.declare_dram_parameter(f"{pre}_{nm}", [L, D], f32, isOutput=False)
    w_in["ff_w1"] = nc.declare_dram_parameter("ff_w1", [L, D, F], bf16, isOutput=False)
    w_in["ff_b1"] = nc.declare_dram_parameter("ff_b1", [L, F], f32, isOutput=False)
    w_in["ff_w2b"] = nc.declare_dram_parameter("ff_w2b", [L, F, D], bf16, isOutput=False)
    w_in["ff_b2"] = nc.declare_dram_parameter("ff_b2", [L, D], f32, isOutput=False)
    if ln_affine:
        for i in (1, 2, 3):
            w_in[f"ln{i}_g"] = nc.declare_dram_parameter(f"ln{i}_g", [L, D], f32, isOutput=False)
            w_in[f"ln{i}_b"] = nc.declare_dram_parameter(f"ln{i}_b", [L, D], f32, isOutput=False)
    onesb_in = nc.declare_dram_parameter("onesb", [P, P], bf16, isOutput=False)
    dmask_in = nc.declare_dram_parameter("dmask", [P, 4, TCH], bf16, isOutput=False)
    pbias_in = nc.declare_dram_parameter("pbias", [P, 12], f32, isOutput=False)
    out_p = nc.declare_dram_parameter("out", [P, KC, HALF], f32, isOutput=True)

    with tile.TileContext(nc, num_cores=8) as tc:
        import contextlib

        gctx = contextlib.ExitStack()
        with gctx:
            persist = gctx.enter_context(tc.tile_pool(name="persist", bufs=1))
            wpool = gctx.enter_context(tc.tile_pool(name="wpool", bufs=1))
            lpool = gctx.enter_context(tc.tile_pool(name="lpool", bufs=1))
            psS = gctx.enter_context(tc.tile_pool(name="psS", bufs=2, space="PSUM"))
            psO = gctx.enter_context(tc.tile_pool(name="psO", bufs=1, space="PSUM"))
            psA = gctx.enter_context(tc.tile_pool(name="psA", bufs=2, space="PSUM"))
            dramp = gctx.enter_context(tc.tile_pool(name="dramp", bufs=2, space="DRAM"))

            # ---- persistent state (SBUF) ----
            hbx = persist.tile([P, KC, HALF], bf16, name="hbx")  # own residual
            kT = persist.tile([P, KC, S], bf16, name="kT")       # shared SA/CA K^T
            kT_c = kT
            vaug = persist.tile([P, NKT, H, AVW], bf16, name="vaug")  # shared aug-V
            oT_s = persist.tile([P, KC, HALF], bf16, name="oT_s")
            oT_c = persist.tile([P, KC, HALF], bf16, name="oT_c")
            x1b = persist.tile([P, KC, HALF], bf16, name="x1b")
            yT = persist.tile([P, KC, HALF], bf16, name="yT")
            h1 = persist.tile([P, NFT, TCH], bf16, name="h1")
            u_t = persist.tile([P, KC, TCH], bf16, name="u_t")   # psum-evac target

            onesb_sb = persist.tile([P, P], bf16, name="onesb_sb")
            dmask_sb = persist.tile([P, 4, TCH], bf16, name="dmask_sb")
            pbias_sb = persist.tile([P, 12], f32, name="pbias_sb")
            zero_sb = persist.tile([P, 1], f32, name="zero_sb")
            eps_sb = persist.tile([P, 1], f32, name="eps_sb")
            nc.vector.memset(zero_sb, 0.0)
            nc.vector.memset(eps_sb, EPS)
            # ones columns of the augmented-V layout, set once (V writes
            # never touch them, across all layers and both attentions)
            nc.vector.memset(vaug[:, :, :, DK:DK + 1], 1.0)

            for kc in range(KC):
                nc.sync.dma_start(out=hbx[:, kc, :], in_=xTb[:, kc, 0:HALF])
            nc.sync.dma_start(out=onesb_sb, in_=onesb_in[:, :])
            nc.sync.dma_start(out=dmask_sb, in_=dmask_in[:, :, :])
            nc.sync.dma_start(out=pbias_sb, in_=pbias_in[:, :])

            pid = nc.sync.partition_id()
            peer = (pid + 1) % 2

            def load_w(dram_t, l, cols, tag, bufs=2, dt=bf16):
                n = dram_t.shape[1] // P
                l = l % L
                t = wpool.tile([P, n, cols], dt, tag=tag, bufs=bufs, name=tag)
                for kc in range(n):
                    nc.sync.dma_start(out=t[:, kc, :], in_=dram_t[l, kc * P:(kc + 1) * P, :])
                return t

            def load_b(dram_t, l, tag):
                n = dram_t.shape[1] // P
                l = l % L
                t = wpool.tile([P, n], f32, tag=tag, bufs=2, name=tag)
                nc.sync.dma_start(out=t, in_=dram_t[l].rearrange("(c p) -> p c", p=P))
                return t

            def evac(dst, src_ps, bias_col, eng):
                if eng == "act":
                    nc.scalar.activation(dst, src_ps, AF.Identity, bias=bias_col)
                else:
                    nc.vector.tensor_scalar(dst, src_ps, bias_col, None, ALU.add)

            # ---------------- building blocks ----------------

            def k_chunk(src, t, wk_sb, bk_sb, kT_t, eng):
                for ft in range(KC):
                    k_ps = psA.tile([P, TCH], f32, tag="acc", name="k_ps")
                    for kc in range(KC):
                        nc.tensor.matmul(k_ps, wk_sb[:, kc, ft * P:(ft + 1) * P],
                                         src[:, kc, :], start=(kc == 0), stop=(kc == KC - 1))
                    evac(kT_t[:, ft, t * TCH:(t + 1) * TCH], k_ps, bk_sb[:, ft:ft + 1], eng)

            def kv_chunk(src, t, wk_sb, bk_sb, wv_sb, bv_sb, eng):
                """K^T + augmented V for one 512-token chunk t (bf16)."""
                k_chunk(src, t, wk_sb, bk_sb, kT, eng)
                for tl in range(4):
                    tt = t * 4 + tl
                    v_ps = psA.tile([P, D], f32, tag="acc", name="v_ps")
                    nmm = KC + (1 if v_bias else 0)
                    for kc in range(KC):
                        nc.tensor.matmul(v_ps, src[:, kc, tl * P:(tl + 1) * P],
                                         wv_sb[:, kc, :], start=(kc == 0),
                                         stop=(kc == nmm - 1))
                    if v_bias:
                        nc.tensor.matmul(v_ps, onesb_sb[0:1, :], bv_sb, start=False, stop=True)
                    # one strided copy drops all 8 heads into the aug layout
                    nc.vector.tensor_copy(
                        out=vaug[:, tt, :, 0:DK],
                        in_=v_ps.rearrange("p (h d) -> p h d", h=H))

            def q_pair(srcT, j, hp, wq_sb, bq_sb, eng):
                """One head pair's Q for query chunk j -> transient tile."""
                qp = lpool.tile([P, TCH], bf16, tag="qp", bufs=1, name="qp")
                q_ps = psA.tile([P, TCH], f32, tag="acc", name="q_ps")
                for kc in range(KC):
                    nc.tensor.matmul(q_ps, wq_sb[:, kc, hp * P:(hp + 1) * P],
                                     srcT[:, kc, j * TCH:(j + 1) * TCH],
                                     start=(kc == 0), stop=(kc == KC - 1))
                evac(qp, q_ps, bq_sb[:, hp:hp + 1], eng)
                return qp

            def attn_pair_j(hp, j, kts, q_src, kT_t, oT_t, fillers):
                """Attention for head pair (2hp, 2hp+1), query chunk j.

                Per k-tile: the two heads' score matmuls target partition
                offsets 0/64 (disjoint PE row groups -> concurrent on HW)
                and land in the two banks of one [P, 2, TCH] f32 PSUM
                tile; a single batched exp covers both heads."""
                hA, hB = 2 * hp, 2 * hp + 1
                qp = q_src(hp, j)
                o_ps = {}
                for hx in (0, 1):
                    o_ps[hx] = psO.tile([AVW, TCH], f32, tag=f"o{hx}", name=f"o_ps{hx}")
                nkt_total = len(kts)
                for done, (kt, bcol, diag) in enumerate(kts):
                    s_ps = psS.tile([P, 2, TCH], f32, tag="s", name="s_ps")
                    pt = lpool.tile([P, 2, TCH], bf16, tag="pt", bufs=3, name="pt")
                    for hx, h in ((0, hA), (1, hB)):
                        off = (h % 2) * DK
                        nc.tensor.matmul(
                            s_ps[:, hx, :],
                            kT_t[off:off + DK, h // 2, kt * P:(kt + 1) * P],
                            qp[off:off + DK, :],
                            start=True, stop=True)
                    bias = zero_sb[:, 0:1] if bcol is None else pbias_sb[:, bcol:bcol + 1]
                    nc.scalar.activation(pt, s_ps, AF.Exp, bias=bias, scale=SCALE)
                    if diag is not None:
                        # all-bf16 SBUF operands -> DVE 4x mode (~200ns)
                        for hx in (0, 1):
                            nc.vector.tensor_mul(pt[:, hx, :], pt[:, hx, :],
                                                 dmask_sb[:, diag, :])
                    for hx, h in ((0, hA), (1, hB)):
                        nc.tensor.matmul(o_ps[hx], vaug[:, kt, h, :],
                                         pt[:, hx, :], start=(done == 0),
                                         stop=(done == nkt_total - 1))
                    if done % 2 == 1:
                        fillers.pop(1)
                # softmax normalization: fast reciprocal + partition broadcast
                for hx, h in ((0, hA), (1, hB)):
                    off = (h % 2) * DK
                    srow = lpool.tile([1, TCH], f32, tag="srow", bufs=1, name="srow")
                    nc.vector.tensor_copy(out=srow, in_=o_ps[hx][DK:AVW, :])
                    rcp = lpool.tile([1, TCH], f32, tag="rcp", bufs=1, name="rcp")
                    nc.vector.reciprocal_approx_fast(out=rcp, in_=srow)
                    rb = lpool.tile([DK, TCH], f32, tag="rb", bufs=1, name="rb")
                    nc.gpsimd.partition_broadcast(rb, rcp, channels=DK)
                    nc.vector.tensor_mul(
                        oT_t[off:off + DK, h // 2, j * TCH:(j + 1) * TCH],
                        o_ps[hx][0:DK, :], rb)

            def out_proj_ft(j, ft, oT_t, wo_sb, bo_sb, eng):
                u_ps = psA.tile([P, TCH], f32, tag="acc", name="u_ps")
                for kc in range(KC):
                    nc.tensor.matmul(u_ps, wo_sb[:, kc, ft * P:(ft + 1) * P],
                                     oT_t[:, kc, j * TCH:(j + 1) * TCH],
                                     start=(kc == 0), stop=(kc == KC - 1))
                # u = wo . oT + h residual, fused on DVE
                nc.vector.scalar_tensor_tensor(
                    out=u_t[:, ft, :], in0=u_ps, scalar=1.0,
                    in1=hbx[:, ft, j * TCH:(j + 1) * TCH],
                    op0=ALU.mult, op1=ALU.add)
                if attn_obias:
                    nc.vector.tensor_scalar(u_t[:, ft, :], u_t[:, ft, :],
                                            bo_sb[:, ft:ft + 1], None, ALU.add)

            def layernorm_t(dsts, g_sb, b_sb):
                """LN over u_t -> dsts(kc)."""
                usq = lpool.tile([P, KC, TCH], bf16, tag="usq", bufs=1, name="usq")
                for kc in range(KC):
                    eng = nc.vector if kc % 2 == 0 else nc.gpsimd
                    eng.tensor_mul(usq[:, kc, :], u_t[:, kc, :], u_t[:, kc, :])
                m_ps = psA.tile([P, TCH], f32, tag="acc", name="m_ps")
                for kc in range(KC):
                    nc.tensor.matmul(m_ps, onesb_sb, u_t[:, kc, :],
                                     start=(kc == 0), stop=(kc == KC - 1))
                q_ps = psA.tile([P, TCH], f32, tag="acc", name="q_ps")
                for kc in range(KC):
                    nc.tensor.matmul(q_ps, onesb_sb, usq[:, kc, :],
                                     start=(kc == 0), stop=(kc == KC - 1))
                t_sb = lpool.tile([P, TCH], f32, tag="lnr", bufs=3, name="t_sb")
                nc.vector.tensor_scalar(t_sb, m_ps, 1.0 / D, None, ALU.mult)
                tt2 = lpool.tile([P, TCH], f32, tag="lnr", bufs=3, name="tt2")
                nc.gpsimd.tensor_mul(tt2, t_sb, t_sb)
                m2 = lpool.tile([P, TCH], f32, tag="lnr", bufs=3, name="m2")
                # m2 = q/D - tt2 in one fused op
                nc.vector.scalar_tensor_tensor(
                    out=m2, in0=q_ps, scalar=1.0 / D, in1=tt2,
                    op0=ALU.mult, op1=ALU.subtract)
                nc.scalar.activation(m2, m2, AF.Ln, bias=eps_sb[:, 0:1])
                r_sb = lpool.tile([P, TCH], f32, tag="lnr", bufs=3, name="r_sb")
                nc.scalar.activation(r_sb, m2, AF.Exp, scale=-0.5, bias=zero_sb[:, 0:1])
                c_sb = lpool.tile([P, TCH], f32, tag="lnc", bufs=2, name="c_sb")
                nc.vector.tensor_mul(c_sb, t_sb, r_sb)
                for kc in range(KC):
                    tmp = lpool.tile([P, TCH], f32, tag="ltmp", bufs=1, name="ltmp")
                    nc.gpsimd.tensor_sub(tmp, u_t[:, kc, :], c_sb)
                    d = dsts(kc)
                    nc.vector.tensor_mul(d, tmp, r_sb)
                    if ln_affine:
                        nc.vector.tensor_scalar(d, d, g_sb[:, kc:kc + 1], b_sb[:, kc:kc + 1],
                                                ALU.mult, ALU.add)

            # ---------------- layer loop ----------------
            pending_fetch = []
            for l in range(NLAYERS):
                last = l == NLAYERS - 1

                # ---- SA projections (ACT idle here -> act evacs) ----
                wk_sa = load_w(w_in["sa_wk"], l, D, "wk", bufs=2)
                wv_sa = load_w(w_in["sa_wv"], l, D, "wv")
                bk_sa = load_b(w_in["sa_bk"], l, "bk")
                bq_sa = load_b(w_in["sa_bq"], l, "bq")
                bv_sa = None
                if v_bias:
                    bv_sa = wpool.tile([1, D], f32, tag="bv", bufs=2, name="bv")
                    nc.sync.dma_start(out=bv_sa, in_=w_in["sa_bv"][l % L:l % L + 1, :])
                wq_sa = load_w(w_in["sa_wq"], l, D, "wq")

                def hbx_chunk(t):
                    return hbx[:, :, t * TCH:(t + 1) * TCH]

                def enc_chunk(t):
                    ec = lpool.tile([P, KC, TCH], bf16, tag="peer", bufs=2,
                                    name="encC")
                    for kc in range(KC):
                        nc.sync.dma_start(out=ec[:, kc, :],
                                          in_=encTb[:, kc, t * TCH:(t + 1) * TCH])
                    return ec

                def peer_chunk(half):
                    pc = lpool.tile([P, KC, TCH], bf16, tag="peer", bufs=2,
                                    name="peerC")
                    if l == 0:
                        for kc in range(KC):
                            nc.sync.dma_start(
                                out=pc[:, kc, :],
                                in_=xTb[:, kc, HALF + half * TCH:HALF + (half + 1) * TCH])
                    else:
                        ccout = pending_fetch[half]
                        nc.sync.dma_start(out=pc, in_=ccout[ts(peer, P), :, :])
                    return pc

                # own chunk t0 + peer chunk j0 only: attention j0 touches
                # just these, so it can start while the prev layer's LN3-t1
                # tail and gather half 1 are still in flight
                kv_chunk(hbx_chunk(0), 0, wk_sa, bk_sa, wv_sa, bv_sa, "act")
                kv_chunk(peer_chunk(0), 2, wk_sa, bk_sa, wv_sa, bv_sa, "act")

                wo_sa = load_w(w_in["sa_wo"], l, D, "wo")
                bo_sa = load_b(w_in["sa_bo"], l, "bo")
                g1 = load_b(w_in["ln1_g"], l, "g1") if ln_affine else None
                b1l = load_b(w_in["ln1_b"], l, "b1l") if ln_affine else None
                wq_ca = load_w(w_in["ca_wq"], l, D, "wq2")
                bq_ca = load_b(w_in["ca_bq"], l, "bq2")

                # ---- SA attention ----
                # j0 touches only own chunks + peer j0 (gather half 0), so it
                # runs while gather half 1 is still in flight; peer chunk j1
                # projection follows it.
                fill = FillerQueue()
                def sa_q(hp, j):
                    return q_pair(hbx, j, hp, wq_sa, bq_sa, "dve")

                for hp in range(4):
                    attn_pair_j(hp, 0, SA_KTS[0], sa_q, kT, oT_s, fill)
                # own chunk t1 (waits on the prev layer's LN3-t1 tail)
                kv_chunk(hbx_chunk(1), 1, wk_sa, bk_sa, wv_sa, bv_sa, "act")
                kv_chunk(peer_chunk(1), 3, wk_sa, bk_sa, wv_sa, bv_sa, "act")
                pending_fetch.clear()
                # fillers for j1: out_proj j0 + LN1 t0
                for ft in range(KC):
                    fill.add(lambda ft=ft: out_proj_ft(0, ft, oT_s, wo_sa, bo_sa, "dve"))
                fill.add(lambda: layernorm_t(
                    lambda kc: x1b[:, kc, 0 * TCH:1 * TCH], g1, b1l))
                for hp in range(4):
                    attn_pair_j(hp, 1, SA_KTS[1], sa_q, kT, oT_s, fill)
                fill.drain()

                # ---- CA K/V from enc (shared kT/vaug) ----
                wk_ca = load_w(w_in["ca_wk"], l, D, "wk")
                bk_ca = load_b(w_in["ca_bk"], l, "bk")
                for t in range(4):
                    k_chunk(enc_chunk(t), t, wk_ca, bk_ca, kT_c, "dve")
                wv_ca = load_w(w_in["ca_wv"], l, D, "wv")
                bv_ca = None
                if v_bias:
                    bv_ca = wpool.tile([1, D], f32, tag="bv", bufs=2, name="bv")
                    nc.sync.dma_start(out=bv_ca, in_=w_in["ca_bv"][l % L:l % L + 1, :])
                for t in range(4):
                    ec = enc_chunk(t)
                    for tl in range(4):
                        tt = t * 4 + tl
                        v_ps = psA.tile([P, D], f32, tag="acc", name="v_ps")
                        nmm = KC + (1 if v_bias else 0)
                        for kc in range(KC):
                            nc.tensor.matmul(v_ps, ec[:, kc, tl * P:(tl + 1) * P],
                                             wv_ca[:, kc, :], start=(kc == 0),
                                             stop=(kc == nmm - 1))
                        if v_bias:
                            nc.tensor.matmul(v_ps, onesb_sb[0:1, :], bv_ca,
                                             start=False, stop=True)
                        nc.vector.tensor_copy(
                            out=vaug[:, tt, :, 0:DK],
                            in_=v_ps.rearrange("p (h d) -> p h d", h=H))

                wo_ca = load_w(w_in["ca_wo"], l, D, "wo")
                bo_ca = load_b(w_in["ca_bo"], l, "bo")
                g2 = load_b(w_in["ln2_g"], l, "g2") if ln_affine else None
                b2l = load_b(w_in["ln2_b"], l, "b2l") if ln_affine else None

                # ---- CA attention ----
                def ca_q(hp, j):
                    return q_pair(x1b, j, hp, wq_ca, bq_ca, "dve")

                fill = FillerQueue()
                # fillers for j0: SA out_proj j1 + LN1 t1
                for ft in range(KC):
                    fill.add(lambda ft=ft: out_proj_ft(1, ft, oT_s, wo_sa, bo_sa, "dve"))
                fill.add(lambda: layernorm_t(
                    lambda kc: x1b[:, kc, 1 * TCH:2 * TCH], g1, b1l))
                for hp in range(4):
                    attn_pair_j(hp, 0, CA_KTS, ca_q, kT_c, oT_c, fill)
                fill.drain()

                w1_sb = load_w(w_in["ff_w1"], l, F, "w1", bufs=1)
                b1_sb = load_b(w_in["ff_b1"], l, "b1")
                w2_sb = wpool.tile([P, NFT, D], bf16, tag="w2", bufs=1, name="w2_sb")
                for kc in range(NFT):
                    nc.sync.dma_start(out=w2_sb[:, kc, :],
                                      in_=w_in["ff_w2b"][l % L, kc * P:(kc + 1) * P, :])
                b2_sb = load_b(w_in["ff_b2"], l, "b2")
                g3 = load_b(w_in["ln3_g"], l, "g3") if ln_affine else None
                b3l = load_b(w_in["ln3_b"], l, "b3l") if ln_affine else None

                def ca_post_j(j):
                    for ft in range(KC):
                        out_proj_ft(j, ft, oT_c, wo_ca, bo_ca, "dve")
                    layernorm_t(lambda kc, j=j: yT[:, kc, j * TCH:(j + 1) * TCH],
                                g2, b2l)

                def ffn1_ft(t, ft):
                    f_ps = psA.tile([P, TCH], f32, tag="acc", name="f_ps")
                    for kc in range(KC):
                        nc.tensor.matmul(f_ps, w1_sb[:, kc, ft * P:(ft + 1) * P],
                                         yT[:, kc, t * TCH:(t + 1) * TCH],
                                         start=(kc == 0), stop=(kc == KC - 1))
                    nc.vector.tensor_scalar(h1[:, ft, :], f_ps,
                                            b1_sb[:, ft:ft + 1], 0.0, ALU.add, ALU.max)

                def ffn1_t(t):
                    for ft in range(NFT):
                        ffn1_ft(t, ft)

                def ffn2_ft(t, ft):
                    g_ps = psA.tile([P, TCH], f32, tag="acc", name="g_ps")
                    for kc in range(NFT):
                        nc.tensor.matmul(g_ps, w2_sb[:, kc, ft * P:(ft + 1) * P],
                                         h1[:, kc, :],
                                         start=(kc == 0), stop=(kc == NFT - 1))
                    # u = ffn2 + x1 residual, fused on DVE
                    nc.vector.scalar_tensor_tensor(
                        out=u_t[:, ft, :], in0=g_ps, scalar=1.0,
                        in1=x1b[:, ft, t * TCH:(t + 1) * TCH],
                        op0=ALU.mult, op1=ALU.add)
                    if ffn_bias:
                        nc.vector.tensor_scalar(u_t[:, ft, :], u_t[:, ft, :],
                                                b2_sb[:, ft:ft + 1], None, ALU.add)

                def ffn2_t(t):
                    for ft in range(KC):
                        ffn2_ft(t, ft)

                def ln3_t(t):
                    if last:
                        # write LN output in place over u_t, then cast-DMA out
                        # (only gpsimd-initiated DMAs can cast bf16 -> f32)
                        layernorm_t(lambda kc: u_t[:, kc, :], g3, b3l)
                        nc.gpsimd.dma_start(out=out_p[:, :, t * TCH:(t + 1) * TCH],
                                            in_=u_t)
                    else:
                        layernorm_t(lambda kc, t=t: hbx[:, kc, t * TCH:(t + 1) * TCH],
                                    g3, b3l)

                def issue_gather(half):
                    """Start the AllGather for own chunk `half`; the peer's
                    copy is fetched into a transient tile next layer."""
                    ccin = dramp.tile([P, KC, TCH], bf16, tag="ccin", bufs=2,
                                      name="ccin")
                    ccout = dramp.tile([2 * P, KC, TCH], bf16, tag="ccout", bufs=2,
                                       name="ccout")
                    nc.sync.dma_start(out=ccin, in_=hbx[:, :, half * TCH:(half + 1) * TCH])
                    nc.gpsimd.collective_compute(
                        "AllGather", ALU.bypass, replica_groups=RG,
                        ins=[ccin.opt()], outs=[ccout.opt()])
                    return ccout

                fill = FillerQueue()
                for ft in range(KC):
                    fill.add(lambda ft=ft: out_proj_ft(0, ft, oT_c, wo_ca, bo_ca, "dve"))
                fill.add(lambda: layernorm_t(
                    lambda kc: yT[:, kc, 0 * TCH:1 * TCH], g2, b2l))
                for ft in range(NFT):
                    fill.add(lambda ft=ft: ffn1_ft(0, ft))
                # chunk-0 FFN2/LN3 + first gather go in as fillers too, so the
                # collective starts as early as possible
                for ft in range(KC):
                    fill.add(lambda ft=ft: ffn2_ft(0, ft))
                fill.add(lambda: ln3_t(0))
                if not last:
                    fill.add(lambda: pending_fetch.append(issue_gather(0)))
                for hp in range(4):
                    attn_pair_j(hp, 1, CA_KTS, ca_q, kT_c, oT_c, fill)
                fill.drain()

                ca_post_j(1)
                ffn1_t(1)
                ffn2_t(1)
                ln3_t(1)
                if not last:
                    pending_fetch.append(issue_gather(1))
                    assert len(pending_fetch) == 2

    nc.finalize()
    return nc


_BUILD_CACHE = {}
LAST_RESULTS = None


def _get_nc(ln_affine, v_bias, ffn_bias=False, attn_obias=False):
    key = (ln_affine, v_bias, ffn_bias, attn_obias, NLAYERS, OPT_ACTSET)
    if key not in _BUILD_CACHE:
        _BUILD_CACHE[key] = build(ln_affine, v_bias, ffn_bias, attn_obias)
    return _BUILD_CACHE[key]


def _to_T(a, dtype):  # [S, D] -> [P, KC, S] feature-major
    return np.ascontiguousarray(a.T.reshape(KC, P, S).transpose(1, 0, 2)).astype(dtype)


def prepare(inputs):
    """Returns (nc, in_maps) for the given full inputs."""
    inp = {k: np.asarray(v) for k, v in inputs.items()}

    ln_affine = not all(
        np.all(inp[f"ln{i}_g"] == 1.0) and np.all(inp[f"ln{i}_b"] == 0.0) for i in (1, 2, 3)
    )
    v_bias = not (np.all(inp["sa_bv"] == 0.0) and np.all(inp["ca_bv"] == 0.0))
    ffn_bias = not np.all(inp["ff_b2"] == 0.0)
    attn_obias = not (np.all(inp["sa_bo"] == 0.0) and np.all(inp["ca_bo"] == 0.0))
    nc = _get_nc(ln_affine, v_bias, ffn_bias, attn_obias)

    ident = np.eye(P, dtype=np.float32)
    pcol = np.arange(P)[:, None]
    qcol = np.arange(TCH)[None, :]
    dmask = np.stack(
        [(qcol >= i * P + pcol) for i in range(4)], axis=1
    ).astype(ml_dtypes.bfloat16)  # [P, 4, TCH]

    shared = {}
    for pre in ("sa", "ca"):
        for nm in ("bq", "bk", "bv", "bo"):
            shared[f"{pre}_{nm}"] = np.ascontiguousarray(inp[f"{pre}_{nm}"], np.float32)
        for nm in ("wq", "wk", "wv"):
            shared[f"{pre}_{nm}"] = inp[f"{pre}_{nm}"].astype(ml_dtypes.bfloat16)
        shared[f"{pre}_wo"] = inp[f"{pre}_wo"].astype(ml_dtypes.bfloat16)
    shared["ff_w1"] = inp["ff_w1"].astype(ml_dtypes.bfloat16)
    shared["ff_b1"] = np.ascontiguousarray(inp["ff_b1"], np.float32)
    shared["ff_w2b"] = inp["ff_w2"].astype(ml_dtypes.bfloat16)
    shared["ff_b2"] = np.ascontiguousarray(inp["ff_b2"], np.float32)
    if ln_affine:
        for i in (1, 2, 3):
            shared[f"ln{i}_g"] = np.ascontiguousarray(inp[f"ln{i}_g"], np.float32)
            shared[f"ln{i}_b"] = np.ascontiguousarray(inp[f"ln{i}_b"], np.float32)
    shared["onesb"] = np.ones((P, P), np.float32).astype(ml_dtypes.bfloat16)
    shared["dmask"] = dmask

    in_maps = []
    for r in range(8):
        b, g = r // 2, r % 2
        mine = [0, 3] if g == 0 else [1, 2]
        theirs = [1, 2] if g == 0 else [0, 3]
        perm = mine + theirs
        xt = np.concatenate([inp["x"][b].T[:, c * TCH:(c + 1) * TCH] for c in perm], axis=1)
        xt = np.ascontiguousarray(xt.reshape(KC, P, S).transpose(1, 0, 2))
        m = dict(shared)
        m["xTb"] = xt.astype(ml_dtypes.bfloat16)
        m["encTb"] = _to_T(np.asarray(inp["enc"][b], np.float32), ml_dtypes.bfloat16)
        pb = np.zeros(12, np.float32)
        # exp-bias columns: j0 kt8-11 -> 0..3 ; j1 kt8-11 -> 4..7 ; j1 kt12-15 -> 8..11
        # Each group of 4 k-tiles lies in one peer global chunk kg; keep iff kg < qg.
        for base, j, kg in ((0, 0, theirs[0]), (4, 1, theirs[0]), (8, 1, theirs[1])):
            pb[base:base + 4] = 0.0 if kg < mine[j] else NEG
        m["pbias"] = np.broadcast_to(pb, (P, 12)).astype(np.float32).copy()
        in_maps.append(m)
    return nc, in_maps


def unshard(results):
    out = np.zeros((B, S, D), np.float32)
    for r in range(8):
        b, g = r // 2, r % 2
        mine = [0, 3] if g == 0 else [1, 2]
        half = results[r]["out"].transpose(1, 0, 2).reshape(D, HALF)
        for j, c in enumerate(mine):
            out[b, c * TCH:(c + 1) * TCH, :] = half[:, j * TCH:(j + 1) * TCH].T
    return out


def kernel(**inputs):
    global LAST_RESULTS
    nc, in_maps = prepare(inputs)

    res = None
    for attempt in range(3):
        try:
            res = run_bass_kernel_spmd(
                nc, in_maps, core_ids=list(range(8)),
                trace=bool(int(os.environ.get("KERNEL_TRACE", "0"))),
            )
            break
        except Exception:
            # first execution after a fresh NEFF compile occasionally flakes
            # on the runtime side; the NEFF cache makes the retry cheap
            if attempt == 2:
                raise
    LAST_RESULTS = res
    return unshard(res.results)
